# revision 34
# baseline (speedup 1.0000x reference)
"""CCBiMambaBlock fused kernel for 8 trn2 NeuronCores.

Sharding: 8 cores = (batch 2) x (direction 2) x (DI-half 2), SPMD (one
program, per-core data). Backward-direction cores receive host-flipped x.
Core map: 0,1 = b0 fwd halves; 2,3 = b1 fwd; 4,5 = b0 bwd; 6,7 = b1 bwd.
The fusion matmul is host-folded into out_proj (M = fusion_w_dir @ out_w), so
mamba_out = sum over (dir, half) of partial projections -> one ReduceScatter
per 4-core batch group, sharding tokens 4-way for the token-parallel tail
(context-clustering, gate, FFN). The token-tail's collective-independent part
(cc path, gate) is emitted early so it fills scan-phase engine idle slots.
"""
import numpy as np
from contextlib import ExitStack

import concourse.bass as bass
import concourse.mybir as mybir
import concourse.tile as tile
from concourse.bass_utils import run_bass_kernel_spmd
from concourse.masks import make_identity

F32 = mybir.dt.float32
F16 = mybir.dt.float16
AL = mybir.AluOpType
AF = mybir.ActivationFunctionType
AX = mybir.AxisListType

P = 128
L = 1024          # tokens per batch
D = 512           # d_model
DI = 1024         # d_inner
DH = 512          # DI per core (half)
NST = 16          # d_state
DT_RANK = 32
KCONV = 4
NC_CLUST = 8
TC = 512          # scan time-chunk
NG = 4            # states per n-group
EPS = 1e-5
N_CORES = 8

_CACHED = {}
BUILD_NOIF = False  # timing builds: emit fwd branch only (TimelineSim can't branch)
BUILD_NOCC = False  # timing builds: replace collective with local DMA copy

# pprod n-groups 0..PPROD_DVE_J-1 run on DVE, the rest on Pool, so the DVE
# (which owns the scans) and Pool finish the scan phase together.
PPROD_DVE_J = 2


def _dt(x):
    return np.ascontiguousarray(x, dtype=np.float16)


def _f32(x):
    return np.ascontiguousarray(x, dtype=np.float32)


def split_multi_waits(nc, max_waits=1):
    """This walrus build rejects >1 sync waits per instruction; move excess
    waits onto preceding same-engine NoOps."""
    n = 0
    for fn in nc.m.functions:
        for blk in fn.blocks:
            out = []
            for inst in blk.instructions:
                si = inst.sync_info
                if si is not None and si.on_wait and len(si.on_wait) > max_waits:
                    waits = list(si.on_wait)
                    excess, keep = waits[:-max_waits], waits[-max_waits:]
                    for i, w in enumerate(excess):
                        out.append(mybir.InstNoOp(
                            name=f"{inst.name}-ws{i}", engine=inst.engine,
                            ins=[], outs=[],
                            sync_info=mybir.SyncInfo(on_wait=[w], on_update=[])))
                        n += 1
                    inst.sync_info = mybir.SyncInfo(
                        on_wait=keep, on_update=list(si.on_update))
                out.append(inst)
            blk.instructions = out
    return n


def _build_nc(a_vals=None):
    nc = bass.Bass("TRN2", target_bir_lowering=False, debug=False,
                   num_devices=N_CORES)

    # ---------------- DRAM I/O ----------------
    di = {}

    def inp(name, shape, dtype):
        di[name] = nc.dram_tensor(name, list(shape), dtype, kind="ExternalInput")
        return di[name]

    inp("x_full", (L, D), F32)
    inp("x_tok", (L // 4, D), F32)
    inp("wT_inz", (D, 1536), F16)
    inp("bias_inz", (12, P), F32)
    inp("wT_xproj", (DI, 64), F16)
    inp("wT_dt", (DT_RANK, DH), F16)
    inp("dt_bias", (4, P), F32)
    inp("A_dev", (DH, NST), F32)
    inp("convw", (DI, KCONV), F32)
    inp("convb", (8, P), F32)
    inp("Dp_dev", (4, P), F32)
    inp("wT_out", (DH, D), F16)
    inp("fusion_b", (1, D), F32)
    inp("cc_wT", (D, D), F16)
    inp("ccb", (4, P), F32)
    inp("centers_nT", (D, NC_CLUST), F16)
    inp("centers_dev", (NC_CLUST, D), F16)
    inp("norm1_g", (1, D), F32)
    inp("norm1_b", (1, D), F32)
    inp("ccg", (1, D), F32)
    inp("ccb2", (1, D), F32)
    inp("alpha_col", (P, 1), F32)
    inp("gate_wT", (D, 2), F16)
    inp("gate_b", (1, 2), F32)
    inp("ffn_w1T", (D, 4 * D), F16)
    inp("ffn_b1", (16, P), F32)
    inp("ffn_w2T", (4 * D, D), F16)
    inp("ffn_b2", (1, D), F32)

    out_slice = nc.dram_tensor("out_slice", [L // 4, D], F32, kind="ExternalOutput")

    rs_in = nc.dram_tensor("rs_in", [4, 256, D], F16)
    rs_out = nc.dram_tensor("rs_out", [256, D], F16)
    bc_dram = nc.dram_tensor("bc_dram", [32, L], F16)   # B rows 0:16, C rows 16:32

    RG = [[0, 1, 4, 5], [2, 3, 6, 7]]

    with tile.TileContext(nc) as tc, ExitStack() as top:
        # persistent pools; `mid` closes before the late tail to free SBUF
        mid = top.enter_context(ExitStack())
        pk = top.enter_context(tc.tile_pool(name="keep", bufs=1))

        rowpool = top.enter_context(tc.tile_pool(name="rows", bufs=1))
        ones1f32 = pk.tile([1, P], F32)
        nc.vector.memset(ones1f32[:], 1.0)
        idf16 = pk.tile([P, P], F16)
        make_identity(nc, idf16[:])
        idf32 = pk.tile([16, 16], F32)
        make_identity(nc, idf32[:])

        # token-tail pools (live to the end)
        ptt = top.enter_context(tc.tile_pool(name="ptt", bufs=1))
        pttb = top.enter_context(tc.tile_pool(name="pttb", bufs=2))
        pttps = top.enter_context(tc.tile_pool(name="pttps", bufs=1, space="PSUM"))

        def layer_norm(src, n_tt, pool, poolb, gb=None, out_dtype=F16, tag="ln"):
            """src [P, n_tt, D] -> normalized tile (optionally * g + b)."""
            st6 = poolb.tile([P, n_tt, 6], F32, tag=tag + "_st6", name=tag + "_st6")
            agg = pool.tile([P, n_tt, 2], F32, tag=tag + "_agg", name=tag + "_agg")
            for tt in range(n_tt):
                nc.vector.bn_stats(st6[:, tt, :], src[:, tt, :])
                nc.vector.bn_aggr(agg[:, tt, :], st6[:, tt, :])
            vr = pool.tile([P, n_tt], F32, tag=tag + "_vr", name=tag + "_vr")
            nc.vector.tensor_scalar_add(vr[:], agg[:, :, 1], EPS)
            nc.scalar.sqrt(vr[:], vr[:])
            rs = pool.tile([P, n_tt], F32, tag=tag + "_rs", name=tag + "_rs")
            nc.vector.reciprocal(rs[:], vr[:])
            o = pool.tile([P, n_tt, D], out_dtype, tag=tag + "_o", name=tag + "_o")
            for tt in range(n_tt):
                nc.vector.tensor_scalar(o[:, tt, :], src[:, tt, :],
                                        agg[:, tt, 0:1], rs[:, tt:tt + 1],
                                        AL.subtract, AL.mult)
                if gb is not None:
                    g_bc, b_bc = gb
                    nc.vector.tensor_mul(o[:, tt, :], o[:, tt, :], g_bc[:])
                    nc.vector.tensor_add(o[:, tt, :], o[:, tt, :], b_bc[:])
            return o

        # ================= Phase 1: LN(x) -> xn, transpose =================
        pw = mid.enter_context(tc.tile_pool(name="mid", bufs=1))
        early = ExitStack()
        pxn = early.enter_context(tc.tile_pool(name="pxn", bufs=1))
        xnT = pxn.tile([P, 4, L], F16)      # [d-part, dblk, t]
        with tc.tile_pool(name="ph1", bufs=2) as p1, \
             tc.tile_pool(name="ph1s", bufs=1) as p1s, \
             tc.tile_pool(name="ph1ps", bufs=2, space="PSUM") as p1ps:
            xsb = p1s.tile([P, 8, D], F32, tag="xsb")
            xr = di["x_full"].ap().rearrange("(k p) d -> p k d", p=P)
            for tt in range(8):
                nc.sync.dma_start(xsb[:, tt, :], xr[:, tt, :])
            st6 = p1s.tile([P, 8, 6], F32, tag="st6")
            agg = p1s.tile([P, 8, 2], F32, tag="agg")
            for tt in range(8):
                nc.vector.bn_stats(st6[:, tt, :], xsb[:, tt, :])
                nc.vector.bn_aggr(agg[:, tt, :], st6[:, tt, :])
            var = p1s.tile([P, 8], F32, tag="var")
            nc.vector.tensor_scalar_add(var[:], agg[:, :, 1], EPS)
            nc.scalar.sqrt(var[:], var[:])
            rstd = p1s.tile([P, 8], F32, tag="rstd")
            nc.vector.reciprocal(rstd[:], var[:])
            xn_tok = p1s.tile([P, 8, D], F16, tag="xntok")
            for tt in range(8):
                nc.vector.tensor_scalar(
                    xn_tok[:, tt, :], xsb[:, tt, :],
                    agg[:, tt, 0:1], rstd[:, tt:tt + 1], AL.subtract, AL.mult)
            # transpose on PE (HWDGE transposes pay ~650ns fixed cost each)
            for tt in range(8):
                pst = p1ps.tile([P, 4, P], F16, tag="tps", name="tps")
                for dd in range(4):
                    nc.tensor.transpose(pst[:, dd, :],
                                        xn_tok[:, tt, dd * P:(dd + 1) * P],
                                        idf16[:])
                nc.scalar.copy(xnT[:, :, tt * P:(tt + 1) * P], pst[:])

        # small per-partition params
        dtb_sb = pk.tile([P, 4], F32)
        nc.sync.dma_start(dtb_sb[:], di["dt_bias"].ap().rearrange("m p -> p m"))
        if a_vals is None:
            A_sb = pk.tile([P, 4, NST], F32)
            nc.sync.dma_start(A_sb[:], di["A_dev"].ap().rearrange("(k p) n -> p k n", p=P))
        convw_sb = pk.tile([P, 8, KCONV], F32)
        nc.sync.dma_start(convw_sb[:], di["convw"].ap().rearrange("(k p) t -> p k t", p=P))
        convb_sb = pk.tile([P, 8], F32)
        nc.sync.dma_start(convb_sb[:], di["convb"].ap().rearrange("k p -> p k"))
        Dp_sb = pk.tile([P, 4], F32)
        nc.sync.dma_start(Dp_sb[:], di["Dp_dev"].ap().rearrange("k p -> p k"))
        alpha_sb = pk.tile([P, 1], F32)
        nc.sync.dma_start(alpha_sb[:], di["alpha_col"].ap())
        biasz_sb = pk.tile([P, 12], F32)
        nc.sync.dma_start(biasz_sb[:], di["bias_inz"].ap().rearrange("m p -> p m"))
        ffnb1_sb = pk.tile([P, 16], F32)
        nc.sync.dma_start(ffnb1_sb[:], di["ffn_b1"].ap().rearrange("m p -> p m"))
        ccbias_sb = pk.tile([P, 4], F32)
        nc.sync.dma_start(ccbias_sb[:], di["ccb"].ap().rearrange("m p -> p m"))

        # row vectors for broadcasts
        rows = {}
        for nm in ["norm1_g", "norm1_b", "ccg", "ccb2", "fusion_b", "ffn_b2"]:
            rows[nm] = rowpool.tile([1, D], F32, tag=nm, name="row_" + nm)
            nc.sync.dma_start(rows[nm][:], di[nm].ap())
        rows["gate_b"] = rowpool.tile([1, 2], F32, tag="gate_b", name="row_gate_b")
        nc.sync.dma_start(rows["gate_b"][:], di["gate_b"].ap())

        # broadcast [1,D] rows across partitions via ones-matmul
        bc = {}
        with tc.tile_pool(name="bcps", bufs=2, space="PSUM") as pps:
            for nm in ["norm1_g", "norm1_b", "ccg", "ccb2", "fusion_b", "ffn_b2", "gate_b"]:
                w = rows[nm].shape[1]
                bct = pk.tile([P, w], F32, tag="bc_" + nm, name="bc_" + nm)
                ps = pps.tile([P, 512], F32, tag="bcps")
                nc.tensor.matmul(ps[:, :w], ones1f32[:], rows[nm][:], start=True, stop=True)
                nc.scalar.copy(bct[:], ps[:, :w])
                bc[nm] = bct

        # main weights (DMAs emitted after phase 1 so x loads first)
        winz_sb = pw.tile([P, 4, 1536], F16)
        nc.gpsimd.dma_start(winz_sb[:], di["wT_inz"].ap().rearrange("(k p) m -> p k m", p=P))
        wxp_sb = pw.tile([P, 8, 64], F16)
        nc.gpsimd.dma_start(wxp_sb[:], di["wT_xproj"].ap().rearrange("(k p) m -> p k m", p=P))
        wdt_sb = pw.tile([DT_RANK, DH], F16)
        nc.gpsimd.dma_start(wdt_sb[:], di["wT_dt"].ap())
        wout_sb = pw.tile([P, 4, D], F16)
        nc.gpsimd.dma_start(wout_sb[:], di["wT_out"].ap().rearrange("(k p) m -> p k m", p=P))

        # ========== Phase 2+3 emitters (per time-half th of 512 tokens) ====
        # th=0 runs inline (scan chunk 0 gates on it); th=1, the z-gate rows,
        # and the token-tail head are deferred as closures popped one per scan
        # iteration, filling PE/Act idle slots under the scan.
        xcT = pw.tile([P, 8, L], F16)       # full-DI conv output (permuted order)
        zT = pw.tile([P, 4, L], F16)        # silu(z) for my half
        delta = pw.tile([P, 4, L], F16)
        dtT = pxn.tile([DT_RANK, L], F16)
        p2c = early.enter_context(tc.tile_pool(name="ph2c", bufs=1))
        p2x = early.enter_context(tc.tile_pool(name="ph2x", bufs=1))
        p2ps = early.enter_context(tc.tile_pool(name="ph2ps", bufs=1, space="PSUM"))
        p3b = early.enter_context(tc.tile_pool(name="ph3b", bufs=1))
        xppA = p2x.tile([P, 3 + L], F16, tag="xppA")
        nc.vector.memset(xppA[:, 0:3], 0.0)
        xppB = p2x.tile([P, 3 + L], F16, tag="xppB")
        nc.vector.memset(xppB[:, 0:3], 0.0)

        def em_inproj(mt, th):
            ps = p2ps.tile([P, 512], F32, tag=f"thps{mt % 2}", name="zps")
            for kd in range(4):
                nc.tensor.matmul(
                    ps[:], winz_sb[:, kd, mt * P:(mt + 1) * P],
                    xnT[:, kd, th * 512:(th + 1) * 512],
                    start=(kd == 0), stop=(kd == 3))
            xpp = xppA if mt % 2 == 0 else xppB
            nc.scalar.activation(xpp[:, 3 + th * 512: 3 + (th + 1) * 512], ps[:],
                                 AF.Identity, bias=biasz_sb[:, mt:mt + 1])

        def em_conv(mt, th):
            # depthwise conv on PE: accumulating matmuls with diag(w_k)
            xpp = xppA if mt % 2 == 0 else xppB
            dgw = p2c.tile([P, KCONV, P], F16, tag="dgw")
            for k in range(KCONV):
                nc.vector.tensor_scalar_mul(dgw[:, k, :], idf16[:],
                                            convw_sb[:, mt, k:k + 1])
            cps = p2ps.tile([P, 512], F32, tag=f"thps{mt % 2}", name="cps")
            for k in range(KCONV):
                nc.tensor.matmul(cps[:], dgw[:, k, :],
                                 xpp[:, k + th * 512: k + th * 512 + 512],
                                 start=(k == 0), stop=(k == 3))
            nc.scalar.activation(xcT[:, mt, th * 512:(th + 1) * 512], cps[:],
                                 AF.Silu, bias=convb_sb[:, mt:mt + 1])

        def em_inconv(mt, th):
            em_inproj(mt, th)
            em_conv(mt, th)

        def em_xproj(th):
            ps = p2ps.tile([64, 512], F32, tag="xdps", name="xdps")
            for kd in range(8):
                nc.tensor.matmul(ps[:], wxp_sb[:, kd, :],
                                 xcT[:, kd, th * 512:(th + 1) * 512],
                                 start=(kd == 0), stop=(kd == 7))
            nc.scalar.copy(dtT[:, th * 512:(th + 1) * 512], ps[0:DT_RANK, :])
            bctmp = p3b.tile([32, 512], F16, tag="bctmp")
            nc.scalar.copy(bctmp[:], ps[32:64, :])
            nc.sync.dma_start(bc_dram.ap()[:, th * 512:(th + 1) * 512], bctmp[:])

        def em_delta(m, th):
            ps = p2ps.tile([P, 512], F32, tag=f"thps{m % 2}", name="dtps")
            nc.tensor.matmul(ps[:], wdt_sb[:, m * P:(m + 1) * P],
                             dtT[:, th * 512:(th + 1) * 512],
                             start=True, stop=True)
            esc = p3b.tile([P, 512], F16, tag="esc")
            nc.scalar.activation(esc[:], ps[:], AF.Exp, bias=dtb_sb[:, m:m + 1])
            nc.scalar.activation(delta[:, m, th * 512:(th + 1) * 512],
                                 esc[:], AF.Ln, bias=1.0)

        def em_z(mt, th):
            ps = p2ps.tile([P, 512], F32, tag=f"thps{mt % 2}", name="zzps")
            for kd in range(4):
                nc.tensor.matmul(
                    ps[:], winz_sb[:, kd, mt * P:(mt + 1) * P],
                    xnT[:, kd, th * 512:(th + 1) * 512],
                    start=(kd == 0), stop=(kd == 3))
            nc.scalar.activation(zT[:, mt - 8, th * 512:(th + 1) * 512], ps[:],
                                 AF.Silu, bias=biasz_sb[:, mt:mt + 1])

        def em_toktail():
            # token-tail head (xn slice + cc/gate weight loads + transposes)
            xn_sl = layer_norm(xtok, 2, ptt, pttb,
                               gb=(bc["norm1_g"], bc["norm1_b"]),
                               out_dtype=F16, tag="lnsl")
            tt_tiles["xn_sl"] = xn_sl
            xnsT = ptt.tile([P, 4, 256], F16, tag="xnsT", name="xnsT")
            for tt in range(2):
                pst = pttps.tile([P, 4, P], F16, tag="ps6", name="ttps")
                for dd in range(4):
                    nc.tensor.transpose(pst[:, dd, :],
                                        xn_sl[:, tt, dd * P:(dd + 1) * P],
                                        idf16[:])
                nc.vector.tensor_copy(xnsT[:, :, tt * P:(tt + 1) * P], pst[:])
            tt_tiles["xnsT"] = xnsT

        # th=0 chain inline: scan chunk 0 can start after this
        for mt in range(8):
            em_inconv(mt, 0)
        em_xproj(0)
        for m in range(4):
            em_delta(m, 0)
        for mt in range(8, 12):
            em_z(mt, 0)

        # deferred th=1 work, popped into the scan loop: one slot per (m, ngi)
        # iteration plus one per m-boundary (20 slots per chunk).  Hard
        # deadlines: conv(mt,1) all before xproj(1); delta(m,1) before chunk-1
        # iterations of m; z(mt,1) before chunk-1 ypost of its m.
        A_ = lambda mt: (lambda: em_inproj(mt, 1))
        B_ = lambda mt: (lambda: em_conv(mt, 1))
        Z_ = lambda mt: (lambda: em_z(mt, 1))
        D_ = lambda m: (lambda: em_delta(m, 1))
        deferred = [
            # ch0-m0 iters + end          # ch0-m1
            A_(0), A_(1), B_(0), A_(2), B_(1),
            A_(3), B_(2), A_(4), B_(3), A_(5),
            # ch0-m2                      # ch0-m3
            B_(4), A_(6), B_(5), A_(7), B_(6),
            B_(7), lambda: em_xproj(1), D_(0), D_(1), D_(2),
            # ch1-m0 iters + end
            D_(3), Z_(8), Z_(9), Z_(10), Z_(11),
            em_toktail,
        ]

        # token-tail x slice + small weights (DMA only; compute is deferred)
        tt_tiles = {}
        xtok = ptt.tile([P, 2, D], F32, tag="xtok")
        nc.sync.dma_start(xtok[:], di["x_tok"].ap().rearrange("(k p) d -> p k d", p=P))
        cw_sb = ptt.tile([P, 4, D], F16, tag="ccw")
        nc.gpsimd.dma_start(cw_sb[:], di["cc_wT"].ap().rearrange("(k p) m -> p k m", p=P))
        cnT_sb = ptt.tile([P, 4, NC_CLUST], F16, tag="cnT")
        nc.gpsimd.dma_start(cnT_sb[:], di["centers_nT"].ap().rearrange("(k p) m -> p k m", p=P))
        cent_sb = ptt.tile([NC_CLUST, D], F16, tag="cent")
        nc.gpsimd.dma_start(cent_sb[:], di["centers_dev"].ap())
        gw_sb = ptt.tile([P, 4, 2], F16, tag="gw")
        nc.gpsimd.dma_start(gw_sb[:], di["gate_wT"].ap().rearrange("(k p) m -> p k m", p=P))

        # ================= Phase 4+5: scan, y, out_proj ====================
        # n-sum strategy: pprod partials are accumulated over n on the PE
        # (identity-matmul into PSUM, idle during the scan), with D*xc folded
        # in as a diagonal matmul; dBu stays on DVE, pprod mostly on Pool.
        hprev = pw.tile([P, 4, NST], F16)
        dgD = pw.tile([P, 4, P], F16)       # diag(D) per m-block
        for m in range(4):
            nc.vector.tensor_scalar_mul(dgD[:, m, :], idf16[:], Dp_sb[:, m:m + 1])
        with tc.tile_pool(name="ph4", bufs=3) as p4, \
             tc.tile_pool(name="ph4bc", bufs=2) as p4bc, \
             tc.tile_pool(name="ph4da", bufs=2) as p4da, \
             tc.tile_pool(name="ph4y1", bufs=1) as p4y1, \
             tc.tile_pool(name="ph4ps", bufs=2, space="PSUM") as p4ps, \
             tc.tile_pool(name="ph5ps", bufs=1, space="PSUM") as p5ps:
            n_ch = L // TC
            for ch in range(n_ch):
                t0 = ch * TC
                yTf = p4y1.tile([P, 4, TC], F16, tag="yTf", name="yTf")
                dus = p4y1.tile([P, 4, TC], F16, tag="dus", name="dus")
                outT = p4y1.tile([P, 2, 2 * D], F16, tag="outT", name="outT")
                for m in range(4):
                    psy = p4ps.tile([P, TC], F32, tag="psy", name="psy")
                    nc.vector.tensor_mul(dus[:, m, :],
                                         delta[:, m, t0:t0 + TC],
                                         xcT[:, m, t0:t0 + TC])
                    for ngi in range(NST // NG):
                        nbase = ngi * NG
                        Bb = p4bc.tile([P, NG, TC], F16, tag="Bb")
                        nc.sync.dma_start(
                            Bb[:], bc_dram.ap()[None, nbase:nbase + NG, t0:t0 + TC]
                            .to_broadcast((P, NG, TC)))
                        Cb = p4bc.tile([P, NG, TC], F16, tag="Cb")
                        nc.sync.dma_start(
                            Cb[:], bc_dram.ap()[None, 16 + nbase:16 + nbase + NG, t0:t0 + TC]
                            .to_broadcast((P, NG, TC)))
                        dA = p4da.tile([P, NG, TC], F16, tag="dA")
                        for j in range(NG):
                            if a_vals is not None:
                                nc.scalar.activation(
                                    dA[:, j, :], delta[:, m, t0:t0 + TC], AF.Exp,
                                    scale=float(a_vals[nbase + j]))
                            else:
                                nc.scalar.activation(
                                    dA[:, j, :], delta[:, m, t0:t0 + TC], AF.Exp,
                                    scale=A_sb[:, m, nbase + j:nbase + j + 1])
                        if deferred:
                            deferred.pop(0)()
                        dBu = p4.tile([P, NG, TC], F16, tag="dBu")
                        nc.vector.tensor_tensor(
                            dBu[:], dus[:, m, None, :].to_broadcast((P, NG, TC)),
                            Bb[:], AL.mult)
                        h = p4.tile([P, NG, TC], F16, tag="h")
                        for j in range(NG):
                            init = 0.0 if ch == 0 else hprev[:, m, nbase + j:nbase + j + 1]
                            nc.vector.tensor_tensor_scan(
                                h[:, j, :], dA[:, j, :], dBu[:, j, :], init,
                                AL.mult, AL.add)
                        if ch < n_ch - 1:
                            nc.vector.tensor_copy(hprev[:, m, nbase:nbase + NG],
                                                  h[:, :, TC - 1])
                        pprod = p4.tile([P, NG, TC], F16, tag="pprod", name="pprod")
                        # pprod split DVE:Pool to unload DVE (the scan engine)
                        # while keeping the Pool link short
                        if PPROD_DVE_J > 0:
                            nc.vector.tensor_mul(pprod[:, 0:PPROD_DVE_J, :],
                                                 h[:, 0:PPROD_DVE_J, :],
                                                 Cb[:, 0:PPROD_DVE_J, :])
                        if PPROD_DVE_J < NG:
                            nc.gpsimd.tensor_mul(pprod[:, PPROD_DVE_J:, :],
                                                 h[:, PPROD_DVE_J:, :],
                                                 Cb[:, PPROD_DVE_J:, :])
                        # n-sum on PE: psy += sum_j pprod[:, j, :]
                        for j in range(NG):
                            nc.tensor.matmul(psy[:], idf16[:], pprod[:, j, :],
                                             start=(ngi == 0 and j == 0),
                                             stop=False)
                    # finish psum: += diag(D) @ xc, then gate with silu(z) + flip
                    nc.tensor.matmul(psy[:], dgD[:, m, :], xcT[:, m, t0:t0 + TC],
                                     start=False, stop=True)
                    if BUILD_NOIF:
                        nc.vector.tensor_tensor(yTf[:, m, :], psy[:],
                                                zT[:, m, t0:t0 + TC], AL.mult)
                    else:
                        pid = nc.partition_id()
                        with tc.If(pid >= 4) as cmp:
                            nc.vector.tensor_tensor(
                                yTf[:, m, :], psy[:, ::-1],
                                zT[:, m, t0:t0 + TC][:, ::-1], AL.mult)
                        with cmp.Else():
                            nc.vector.tensor_tensor(yTf[:, m, :], psy[:],
                                                    zT[:, m, t0:t0 + TC], AL.mult)
                    if deferred:
                        deferred.pop(0)()
                # out_proj (token-part output); for backward cores this chunk's
                # yTf holds true tokens [L-t0-TC, L-t0), i.e. chunk (n_ch-1-ch)
                for tt in range(4):
                    ps = p5ps.tile([P, 512], F32, tag="ops")
                    for m in range(4):
                        nc.tensor.matmul(ps[:], yTf[:, m, tt * P:(tt + 1) * P],
                                         wout_sb[:, m, :],
                                         start=(m == 0), stop=(m == 3))
                    nc.scalar.copy(outT[:, tt // 2, (tt % 2) * D:(tt % 2 + 1) * D], ps[:])
                pchs = [2 * ch, 2 * ch + 1]
                if BUILD_NOIF:
                    for p_ch in pchs:
                        for sub in range(2):
                            nc.sync.dma_start(
                                rs_in.ap()[p_ch, sub * P:(sub + 1) * P, :],
                                outT[:, p_ch - 2 * ch, sub * D:(sub + 1) * D])
                else:
                    with tc.If(pid >= 4) as cmp2:
                        for p_ch in pchs:
                            for sub in range(2):
                                nc.sync.dma_start(
                                    rs_in.ap()[p_ch ^ 2, sub * P:(sub + 1) * P, :],
                                    outT[:, p_ch - 2 * ch, sub * D:(sub + 1) * D])
                    with cmp2.Else():
                        for p_ch in pchs:
                            for sub in range(2):
                                nc.sync.dma_start(
                                    rs_in.ap()[p_ch, sub * P:(sub + 1) * P, :],
                                    outT[:, p_ch - 2 * ch, sub * D:(sub + 1) * D])

        early.close()

        # ====== Token-tail part 2: cc path, gate ====
        xn_sl = tt_tiles["xn_sl"]
        xnsT = tt_tiles["xnsT"]
        projT = ptt.tile([P, 4, 256], F16, tag="projT")
        sqT = ptt.tile([P, 4, 256], F16, tag="sqT")
        for pf in range(4):
            ps = pttps.tile([P, 256], F32, tag="ps6")
            for kd in range(4):
                nc.tensor.matmul(ps[:], cw_sb[:, kd, pf * P:(pf + 1) * P],
                                 xnsT[:, kd, :], start=(kd == 0), stop=(kd == 3))
            nc.scalar.activation(projT[:, pf, :], ps[:], AF.Identity,
                                 bias=ccbias_sb[:, pf:pf + 1])
            nc.scalar.activation(sqT[:, pf, :], projT[:, pf, :], AF.Square)
        onescol = ptt.tile([P, 1], F16, tag="onescol")
        nc.vector.memset(onescol[:], 1.0)
        stack = ptt.tile([16, 256], F32, tag="stack")
        nc.vector.memset(stack[:], 0.0)
        ps_sim = pttps.tile([NC_CLUST, 256], F32, tag="pst6", name="ps_sim")
        for kd in range(4):
            nc.tensor.matmul(ps_sim[:], cnT_sb[:, kd, :], projT[:, kd, :],
                             start=(kd == 0), stop=(kd == 3))
        nc.scalar.copy(stack[0:8, :], ps_sim[:])
        ps_ssq = pttps.tile([1, 256], F32, tag="pst6", name="ps_ssq")
        for kd in range(4):
            nc.tensor.matmul(ps_ssq[:], onescol[:], sqT[:, kd, :],
                             start=(kd == 0), stop=(kd == 3))
        ssq_tmp = ptt.tile([1, 256], F32, tag="ssq_tmp")
        nc.scalar.copy(ssq_tmp[:], ps_ssq[:])
        nc.sync.dma_start(stack[8:9, :], ssq_tmp[:])
        S = ptt.tile([P, 2, 16], F32, tag="S")
        for tt in range(2):
            pst = pttps.tile([P, 16], F32, tag="pst6", name="stps")
            nc.tensor.transpose(pst[:], stack[:, tt * P:(tt + 1) * P],
                                idf32[:])
            nc.scalar.copy(S[:, tt, :], pst[:])
        nrm = ptt.tile([P, 2], F32, tag="nrm")
        nc.scalar.sqrt(nrm[:], S[:, :, 8])
        nc.vector.tensor_scalar_max(nrm[:], nrm[:], 1e-12)
        rnrm = ptt.tile([P, 2], F32, tag="rnrm")
        nc.vector.reciprocal(rnrm[:], nrm[:])
        wcl = ptt.tile([P, 2, NC_CLUST], F16, tag="wcl")
        for tt in range(2):
            sim = pttb.tile([P, NC_CLUST], F32, tag="sim")
            nc.vector.tensor_scalar_mul(sim[:], S[:, tt, 0:8], rnrm[:, tt:tt + 1])
            mx = pttb.tile([P, 1], F32, tag="mx")
            nc.vector.tensor_reduce(mx[:], sim[:], AX.X, AL.max)
            nmx = pttb.tile([P, 1], F32, tag="nmx")
            nc.vector.tensor_scalar_mul(nmx[:], mx[:], -1.0)
            se = pttb.tile([P, 1], F32, tag="se")
            ex = pttb.tile([P, NC_CLUST], F32, tag="ex")
            nc.scalar.activation(ex[:], sim[:], AF.Exp, bias=nmx[:], accum_out=se[:])
            rse = pttb.tile([P, 1], F32, tag="rse")
            nc.vector.reciprocal(rse[:], se[:])
            nc.vector.tensor_scalar_mul(wcl[:, tt, :], ex[:], rse[:])
        wclT = ptt.tile([NC_CLUST, 256], F16, tag="wclT")
        for tt in range(2):
            pst = pttps.tile([NC_CLUST, P], F16, tag="pst6", name="wtps")
            nc.tensor.transpose(pst[:], wcl[:, tt, :], idf16[:])
            nc.scalar.copy(wclT[:, tt * P:(tt + 1) * P], pst[:])
        ccpre = ptt.tile([P, 2, D], F32, tag="ccpre")
        for tt in range(2):
            ps = pttps.tile([P, D], F32, tag="ps6", name="ctxps")
            nc.tensor.matmul(ps[:], wclT[:, tt * P:(tt + 1) * P], cent_sb[:],
                             start=True, stop=True)
            nc.vector.scalar_tensor_tensor(ccpre[:, tt, :], ps[:], alpha_sb[:],
                                           xn_sl[:, tt, :], AL.mult, AL.add)
        cc_out = layer_norm(ccpre, 2, ptt, pttb, gb=(bc["ccg"], bc["ccb2"]),
                            out_dtype=F32, tag="lncc")

        gcl = ptt.tile([P, 2, 2], F32, tag="gcl")
        for tt in range(2):
            ps = pttps.tile([P, D], F32, tag="ps6", name="gps")
            for kd in range(4):
                nc.tensor.matmul(ps[:, 0:2], xnsT[:, kd, tt * P:(tt + 1) * P],
                                 gw_sb[:, kd, :], start=(kd == 0), stop=(kd == 3))
            gpre = pttb.tile([P, 2], F32, tag="gpre")
            nc.vector.tensor_add(gpre[:], ps[:, 0:2], bc["gate_b"][:])
            mx = pttb.tile([P, 1], F32, tag="gmx")
            nc.vector.tensor_reduce(mx[:], gpre[:], AX.X, AL.max)
            nmx = pttb.tile([P, 1], F32, tag="gnmx")
            nc.vector.tensor_scalar_mul(nmx[:], mx[:], -1.0)
            se = pttb.tile([P, 1], F32, tag="gse")
            ex = pttb.tile([P, 2], F32, tag="gex")
            nc.scalar.activation(ex[:], gpre[:], AF.Exp, bias=nmx[:], accum_out=se[:])
            rse = pttb.tile([P, 1], F32, tag="grse")
            nc.vector.reciprocal(rse[:], se[:])
            nc.vector.tensor_scalar_mul(gcl[:, tt, :], ex[:], rse[:])

        if BUILD_NOCC:
            nc.sync.dma_start(rs_out.ap(), rs_in.ap()[0])
        else:
            nc.gpsimd.collective_compute(
                "ReduceScatter", AL.add, ins=[rs_in.ap()], outs=[rs_out.ap()],
                replica_groups=RG)
        mid.close()

        # ================= Late tail: fuse + FFN ===========================
        with tc.tile_pool(name="ph6", bufs=1) as p6, \
             tc.tile_pool(name="ph6b", bufs=2) as p6b, \
             tc.tile_pool(name="ph6ps", bufs=2, space="PSUM") as p6ps:
            mamba16 = p6.tile([P, 2, D], F16, tag="mamba16")
            nc.sync.dma_start(mamba16[:], rs_out.ap().rearrange("(k p) d -> p k d", p=P))
            mamba = p6.tile([P, 2, D], F32, tag="mamba")
            nc.vector.tensor_tensor(
                mamba[:], mamba16[:],
                bc["fusion_b"][:, None, :].to_broadcast((P, 2, D)), AL.add)

            # t0c precomputed (gcl/cc_out ready before the collective lands)
            t0c = p6.tile([P, 2, D], F32, tag="t0c")
            for tt in range(2):
                nc.vector.tensor_scalar_mul(t0c[:, tt, :], cc_out[:, tt, :],
                                            gcl[:, tt, 1:2])
                nc.vector.tensor_add(t0c[:, tt, :], t0c[:, tt, :], xtok[:, tt, :])
            x2 = p6.tile([P, 2, D], F32, tag="x2")
            for tt in range(2):
                nc.vector.scalar_tensor_tensor(x2[:, tt, :], mamba[:, tt, :],
                                               gcl[:, tt, 0:1], t0c[:, tt, :],
                                               AL.mult, AL.add)

            hln = layer_norm(x2, 2, p6, p6b, gb=None, out_dtype=F16, tag="lnffn")
            hT = p6.tile([P, 4, 256], F16, tag="hT")
            for tt in range(2):
                pst = p6ps.tile([P, 4, P], F16, tag="ps6", name="htps")
                for dd in range(4):
                    nc.tensor.transpose(pst[:, dd, :],
                                        hln[:, tt, dd * P:(dd + 1) * P],
                                        idf16[:])
                nc.vector.tensor_copy(hT[:, :, tt * P:(tt + 1) * P], pst[:])
            w1_sb = p6.tile([P, 4, 4 * D], F16, tag="w1")
            nc.gpsimd.dma_start(w1_sb[:], di["ffn_w1T"].ap().rearrange("(k p) m -> p k m", p=P))
            w2_sb = p6.tile([P, 16, D], F16, tag="w2")
            nc.gpsimd.dma_start(w2_sb[:], di["ffn_w2T"].ap().rearrange("(k p) m -> p k m", p=P))
            gT = p6.tile([P, 16, 256], F16, tag="gT")
            for gq in range(4):
                ps = p6ps.tile([P, 2, 256], F32, tag="ps6", name="f1ps")
                for gh in range(2):
                    gf = 2 * gq + gh
                    for kd in range(4):
                        nc.tensor.matmul(ps[:, gh, :],
                                         w1_sb[:, kd, gf * P:(gf + 1) * P],
                                         hT[:, kd, :], start=(kd == 0), stop=(kd == 3))
                nc.scalar.activation(gT[:, 2 * gq, :], ps[:, 0, :], AF.Gelu,
                                     bias=ffnb1_sb[:, 2 * gq:2 * gq + 1])
                nc.scalar.activation(gT[:, 2 * gq + 1, :], ps[:, 1, :], AF.Gelu,
                                     bias=ffnb1_sb[:, 2 * gq + 1:2 * gq + 2])
            for gq in range(4, 8):
                ps = p6ps.tile([P, 2, 256], F32, tag="ps6", name="f1ps")
                for gh in range(2):
                    gf = 2 * gq + gh
                    for kd in range(4):
                        nc.tensor.matmul(ps[:, gh, :],
                                         w1_sb[:, kd, gf * P:(gf + 1) * P],
                                         hT[:, kd, :], start=(kd == 0), stop=(kd == 3))
                nc.scalar.activation(gT[:, 2 * gq, :], ps[:, 0, :], AF.Gelu,
                                     bias=ffnb1_sb[:, 2 * gq:2 * gq + 1])
                nc.scalar.activation(gT[:, 2 * gq + 1, :], ps[:, 1, :], AF.Gelu,
                                     bias=ffnb1_sb[:, 2 * gq + 1:2 * gq + 2])
            for tt in range(2):
                ps = p6ps.tile([P, D], F32, tag="ps6", name="f2ps")
                for gf in range(16):
                    nc.tensor.matmul(ps[:], gT[:, gf, tt * P:(tt + 1) * P],
                                     w2_sb[:, gf, :], start=(gf == 0), stop=(gf == 15))
                ot = p6b.tile([P, D], F32, tag="ot")
                nc.vector.tensor_add(ot[:], ps[:], x2[:, tt, :])
                nc.vector.tensor_add(ot[:], ot[:], bc["ffn_b2"][:])
                nc.sync.dma_start(
                    out_slice.ap().rearrange("(k p) d -> p k d", p=P)[:, tt, :], ot[:])

    return nc


def _prep_inputs(inputs):
    """Build the 8 per-core input dicts from the full problem inputs."""
    x = _f32(inputs["x"])
    in_maps = []
    for c in range(N_CORES):
        half = c & 1
        batch = (c >> 1) & 1
        flip = c >= 4
        pos = (c & 1) + 2 * (c >> 2)
        pfx = "bm_" if flip else "fm_"
        g = lambda k: np.asarray(inputs[pfx + k])

        perm = np.r_[half * DH:(half + 1) * DH, (1 - half) * DH:(2 - half) * DH]
        in_w = np.asarray(g("in_w"))          # [2048, 512]
        xp_w = in_w[:DI][perm]
        z_w = in_w[DI + half * DH: DI + (half + 1) * DH]
        W_inz = np.concatenate([xp_w, z_w], axis=0)         # [1536, 512]
        n1g = _f32(inputs["norm1_g"])
        n1b = _f32(inputs["norm1_b"])
        wT_inz = _dt((W_inz * n1g[None, :]).T)
        bias_inz = _f32(W_inz @ n1b).reshape(12, P)

        xproj_w = np.asarray(g("xproj_w"))                  # [64, 1024]
        wT_xproj = _dt(xproj_w[:, perm].T)

        dt_w = np.asarray(g("dt_w"))                        # [1024, 32]
        wT_dt = _dt(dt_w[half * DH:(half + 1) * DH].T)
        dt_bias = _f32(g("dt_b")[half * DH:(half + 1) * DH]).reshape(4, P)

        A = -np.exp(_f32(g("A_log")))
        A_dev = _f32(A[half * DH:(half + 1) * DH])

        convw = _f32(g("conv_w")[:, 0, :][perm])
        convb = _f32(g("conv_b")[perm]).reshape(8, P)
        Dp_dev = _f32(g("D")[half * DH:(half + 1) * DH]).reshape(4, P)

        fusion_w = np.asarray(inputs["fusion_w"])
        # fusion input is concat(f_out, b_out): f -> cols 0:512, b -> 512:1024
        Wdir = fusion_w[:, 512:1024] if flip else fusion_w[:, 0:512]
        M = Wdir @ np.asarray(g("out_w"))                   # [512o, 1024di]
        wT_out = _dt(M[:, half * DH:(half + 1) * DH].T)

        centers = _f32(inputs["cc_centers"])
        cn = centers / np.maximum(np.linalg.norm(centers, axis=-1, keepdims=True), 1e-12)

        d = {
            "x_full": _f32(x[batch, ::-1] if flip else x[batch]),
            "x_tok": _f32(x[batch, pos * 256:(pos + 1) * 256]),
            "wT_inz": wT_inz,
            "bias_inz": bias_inz,
            "wT_xproj": wT_xproj,
            "wT_dt": wT_dt,
            "dt_bias": dt_bias,
            "A_dev": A_dev,
            "convw": convw,
            "convb": convb,
            "Dp_dev": Dp_dev,
            "wT_out": wT_out,
            "fusion_b": _f32(inputs["fusion_b"]).reshape(1, D),
            "cc_wT": _dt(np.asarray(inputs["cc_proj_w"]).T),
            "ccb": _f32(inputs["cc_proj_b"]).reshape(4, P),
            "centers_nT": _dt(cn.T),
            "centers_dev": _dt(centers),
            "norm1_g": n1g.reshape(1, D),
            "norm1_b": n1b.reshape(1, D),
            "ccg": _f32(inputs["cc_norm_g"]).reshape(1, D),
            "ccb2": _f32(inputs["cc_norm_b"]).reshape(1, D),
            "alpha_col": np.full((P, 1), float(np.asarray(inputs["cc_alpha"]).ravel()[0]), np.float32),
            "gate_wT": _dt(np.asarray(inputs["gate_w"]).T),
            "gate_b": _f32(inputs["gate_b"]).reshape(1, 2),
            "ffn_w1T": _dt((np.asarray(inputs["ffn_w1"]) * _f32(inputs["ffn_norm_g"])[None, :]).T),
            "ffn_b1": _f32(np.asarray(inputs["ffn_b1"]) + np.asarray(inputs["ffn_w1"]) @ _f32(inputs["ffn_norm_b"])).reshape(16, P),
            "ffn_w2T": _dt(np.asarray(inputs["ffn_w2"]).T),
            "ffn_b2": _f32(inputs["ffn_b2"]).reshape(1, D),
        }
        in_maps.append(d)
    return in_maps


TRACE = False
LAST_RESULT = {}


def _detect_uniform_A(inputs):
    As = [-np.exp(_f32(np.asarray(inputs[p + "A_log"]))) for p in ("fm_", "bm_")]
    a0 = As[0][0]
    for A in As:
        if not np.allclose(A, a0[None, :], rtol=0, atol=0):
            return None
    return tuple(float(v) for v in a0)


def kernel(**inputs):
    a_vals = _detect_uniform_A(inputs)
    key = ("nc", a_vals)
    if key not in _CACHED:
        nc = _build_nc(a_vals=a_vals)
        split_multi_waits(nc)
        _CACHED[key] = nc
    nc = _CACHED[key]
    in_maps = _prep_inputs(inputs)
    res = run_bass_kernel_spmd(nc, in_maps, core_ids=list(range(N_CORES)),
                               trace=TRACE)
    LAST_RESULT["res"] = res
    out = np.empty((2, L, D), np.float32)
    for c in range(N_CORES):
        batch = (c >> 1) & 1
        pos = (c & 1) + 2 * (c >> 2)
        out[batch, pos * 256:(pos + 1) * 256] = res.results[c]["out_slice"]
    return out



# revision 49
# speedup vs baseline: 1.0966x; 1.0966x over previous
"""CCBiMambaBlock fused kernel for 8 trn2 NeuronCores.

Sharding: 8 cores = (batch 2) x (direction 2) x (DI-half 2), SPMD (one
program, per-core data). Backward-direction cores receive host-flipped x.
Core map: 0,1 = b0 fwd halves; 2,3 = b1 fwd; 4,5 = b0 bwd; 6,7 = b1 bwd.
The fusion matmul is host-folded into out_proj (M = fusion_w_dir @ out_w), so
mamba_out = sum over (dir, half) of partial projections -> one ReduceScatter
per 4-core batch group, sharding tokens 4-way for the token-parallel tail
(context-clustering, gate, FFN). The token-tail's collective-independent part
(cc path, gate) is emitted early so it fills scan-phase engine idle slots.
"""
import numpy as np
from contextlib import ExitStack

import concourse.bass as bass
import concourse.mybir as mybir
import concourse.tile as tile
from concourse.bass_utils import run_bass_kernel_spmd
from concourse.masks import make_identity

F32 = mybir.dt.float32
F16 = mybir.dt.float16
AL = mybir.AluOpType
AF = mybir.ActivationFunctionType
AX = mybir.AxisListType

P = 128
L = 1024          # tokens per batch
D = 512           # d_model
DI = 1024         # d_inner
DH = 512          # DI per core (half)
NST = 16          # d_state
DT_RANK = 32
KCONV = 4
NC_CLUST = 8
TC = 512          # scan time-chunk
NG = 4            # states per n-group
EPS = 1e-5
N_CORES = 8

_CACHED = {}
BUILD_NOIF = False  # timing builds: emit fwd branch only (TimelineSim can't branch)
BUILD_NOCC = False  # timing builds: replace collective with local DMA copy

# pprod n-groups 0..PPROD_DVE_J-1 run on DVE, the rest on Pool, so the DVE
# (which owns the scans) and Pool finish the scan phase together.
PPROD_DVE_J = 2


def _dt(x):
    return np.ascontiguousarray(x, dtype=np.float16)


def _f32(x):
    return np.ascontiguousarray(x, dtype=np.float32)


def split_multi_waits(nc, max_waits=1):
    """This walrus build rejects >1 sync waits per instruction; move excess
    waits onto preceding same-engine NoOps."""
    n = 0
    for fn in nc.m.functions:
        for blk in fn.blocks:
            out = []
            for inst in blk.instructions:
                si = inst.sync_info
                if si is not None and si.on_wait and len(si.on_wait) > max_waits:
                    waits = list(si.on_wait)
                    excess, keep = waits[:-max_waits], waits[-max_waits:]
                    for i, w in enumerate(excess):
                        out.append(mybir.InstNoOp(
                            name=f"{inst.name}-ws{i}", engine=inst.engine,
                            ins=[], outs=[],
                            sync_info=mybir.SyncInfo(on_wait=[w], on_update=[])))
                        n += 1
                    inst.sync_info = mybir.SyncInfo(
                        on_wait=keep, on_update=list(si.on_update))
                out.append(inst)
            blk.instructions = out
    return n


def _build_nc(a_vals=None):
    nc = bass.Bass("TRN2", target_bir_lowering=False, debug=False,
                   num_devices=N_CORES)

    # ---------------- DRAM I/O ----------------
    di = {}

    def inp(name, shape, dtype):
        di[name] = nc.dram_tensor(name, list(shape), dtype, kind="ExternalInput")
        return di[name]

    inp("x_full", (L, D), F32)
    inp("x_tok", (L // 4, D), F32)
    inp("wT_inz", (D, 1536), F16)
    inp("bias_inz", (12, P), F32)
    inp("wT_xproj", (DI, 64), F16)
    inp("wT_dt", (DT_RANK, DH), F16)
    inp("dt_bias", (4, P), F32)
    inp("A_dev", (DH, NST), F32)
    inp("convw", (DI, KCONV), F32)
    inp("convb", (8, P), F32)
    inp("Dp_dev", (4, P), F32)
    inp("wT_out", (DH, D), F16)
    inp("fusion_b", (1, D), F32)
    inp("cc_wT", (D, D), F16)
    inp("ccb", (4, P), F32)
    inp("centers_nT", (D, NC_CLUST), F16)
    inp("centers_dev", (NC_CLUST, D), F16)
    inp("norm1_g", (1, D), F32)
    inp("norm1_b", (1, D), F32)
    inp("ccg", (1, D), F32)
    inp("ccb2", (1, D), F32)
    inp("alpha_col", (P, 1), F32)
    inp("gate_wT", (D, 2), F16)
    inp("gate_b", (1, 2), F32)
    inp("ffn_w1T", (D, 4 * D), F16)
    inp("ffn_b1", (16, P), F32)
    inp("ffn_w2T", (4 * D, D), F16)
    inp("ffn_b2", (1, D), F32)

    out_slice = nc.dram_tensor("out_slice", [L // 4, D], F32, kind="ExternalOutput")

    rs_in = nc.dram_tensor("rs_in", [4, 256, D], F16)
    rs_out = nc.dram_tensor("rs_out", [256, D], F16)
    bc_dram = nc.dram_tensor("bc_dram", [32, L], F16)   # B rows 0:16, C rows 16:32

    RG = [[0, 1, 4, 5], [2, 3, 6, 7]]

    with tile.TileContext(nc) as tc, ExitStack() as top:
        # persistent pools; `mid` closes before the late tail to free SBUF
        mid = top.enter_context(ExitStack())
        pk = top.enter_context(tc.tile_pool(name="keep", bufs=1))

        rowpool = top.enter_context(tc.tile_pool(name="rows", bufs=1))
        ones1f32 = pk.tile([1, P], F32)
        nc.vector.memset(ones1f32[:], 1.0)
        idf16 = pk.tile([P, P], F16)
        make_identity(nc, idf16[:])
        idf32 = pk.tile([16, 16], F32)
        make_identity(nc, idf32[:])

        # token-tail pools (live to the end)
        ptt = top.enter_context(tc.tile_pool(name="ptt", bufs=1))
        pttb = top.enter_context(tc.tile_pool(name="pttb", bufs=2))
        pttps = top.enter_context(tc.tile_pool(name="pttps", bufs=1, space="PSUM"))

        def layer_norm(src, n_tt, pool, poolb, gb=None, out_dtype=F16, tag="ln"):
            """src [P, n_tt, D] -> normalized tile (optionally * g + b)."""
            st6 = poolb.tile([P, n_tt, 6], F32, tag=tag + "_st6", name=tag + "_st6")
            agg = pool.tile([P, n_tt, 2], F32, tag=tag + "_agg", name=tag + "_agg")
            for tt in range(n_tt):
                nc.vector.bn_stats(st6[:, tt, :], src[:, tt, :])
                nc.vector.bn_aggr(agg[:, tt, :], st6[:, tt, :])
            vr = pool.tile([P, n_tt], F32, tag=tag + "_vr", name=tag + "_vr")
            nc.vector.tensor_scalar_add(vr[:], agg[:, :, 1], EPS)
            nc.scalar.sqrt(vr[:], vr[:])
            rs = pool.tile([P, n_tt], F32, tag=tag + "_rs", name=tag + "_rs")
            nc.vector.reciprocal(rs[:], vr[:])
            o = pool.tile([P, n_tt, D], out_dtype, tag=tag + "_o", name=tag + "_o")
            for tt in range(n_tt):
                nc.vector.tensor_scalar(o[:, tt, :], src[:, tt, :],
                                        agg[:, tt, 0:1], rs[:, tt:tt + 1],
                                        AL.subtract, AL.mult)
                if gb is not None:
                    g_bc, b_bc = gb
                    nc.vector.tensor_mul(o[:, tt, :], o[:, tt, :], g_bc[:])
                    nc.vector.tensor_add(o[:, tt, :], o[:, tt, :], b_bc[:])
            return o

        # ================= Phase 1: LN(x) -> xn, transpose =================
        pw = mid.enter_context(tc.tile_pool(name="mid", bufs=1))
        early = ExitStack()
        pxn = early.enter_context(tc.tile_pool(name="pxn", bufs=1))
        xnT = pxn.tile([P, 4, L], F16)      # [d-part, dblk, t]
        with tc.tile_pool(name="ph1", bufs=2) as p1, \
             tc.tile_pool(name="ph1s", bufs=1) as p1s, \
             tc.tile_pool(name="ph1ps", bufs=2, space="PSUM") as p1ps:
            xsb = p1s.tile([P, 8, D], F32, tag="xsb")
            xr = di["x_full"].ap().rearrange("(k p) d -> p k d", p=P)
            for tt in range(8):
                nc.sync.dma_start(xsb[:, tt, :], xr[:, tt, :])
            st6 = p1s.tile([P, 8, 6], F32, tag="st6")
            agg = p1s.tile([P, 8, 2], F32, tag="agg")
            for tt in range(8):
                nc.vector.bn_stats(st6[:, tt, :], xsb[:, tt, :])
                nc.vector.bn_aggr(agg[:, tt, :], st6[:, tt, :])
            var = p1s.tile([P, 8], F32, tag="var")
            nc.vector.tensor_scalar_add(var[:], agg[:, :, 1], EPS)
            nc.scalar.sqrt(var[:], var[:])
            rstd = p1s.tile([P, 8], F32, tag="rstd")
            nc.vector.reciprocal(rstd[:], var[:])
            xn_tok = p1s.tile([P, 8, D], F16, tag="xntok")
            for tt in range(8):
                nc.vector.tensor_scalar(
                    xn_tok[:, tt, :], xsb[:, tt, :],
                    agg[:, tt, 0:1], rstd[:, tt:tt + 1], AL.subtract, AL.mult)
            # transpose on PE (HWDGE transposes pay ~650ns fixed cost each)
            for tt in range(8):
                pst = p1ps.tile([P, 4, P], F16, tag="tps", name="tps")
                for dd in range(4):
                    nc.tensor.transpose(pst[:, dd, :],
                                        xn_tok[:, tt, dd * P:(dd + 1) * P],
                                        idf16[:])
                nc.vector.tensor_copy(xnT[:, :, tt * P:(tt + 1) * P], pst[:])

        # small per-partition params
        dtb_sb = pk.tile([P, 4], F32)
        nc.sync.dma_start(dtb_sb[:], di["dt_bias"].ap().rearrange("m p -> p m"))
        if a_vals is None:
            A_sb = pk.tile([P, 4, NST], F32)
            nc.sync.dma_start(A_sb[:], di["A_dev"].ap().rearrange("(k p) n -> p k n", p=P))
        convw_sb = pk.tile([P, 8, KCONV], F32)
        nc.sync.dma_start(convw_sb[:], di["convw"].ap().rearrange("(k p) t -> p k t", p=P))
        convb_sb = pk.tile([P, 8], F32)
        nc.sync.dma_start(convb_sb[:], di["convb"].ap().rearrange("k p -> p k"))
        Dp_sb = pk.tile([P, 4], F32)
        nc.sync.dma_start(Dp_sb[:], di["Dp_dev"].ap().rearrange("k p -> p k"))
        alpha_sb = pk.tile([P, 1], F32)
        nc.sync.dma_start(alpha_sb[:], di["alpha_col"].ap())
        biasz_sb = pk.tile([P, 12], F32)
        nc.sync.dma_start(biasz_sb[:], di["bias_inz"].ap().rearrange("m p -> p m"))
        ffnb1_sb = pk.tile([P, 16], F32)
        nc.sync.dma_start(ffnb1_sb[:], di["ffn_b1"].ap().rearrange("m p -> p m"))
        ccbias_sb = pk.tile([P, 4], F32)
        nc.sync.dma_start(ccbias_sb[:], di["ccb"].ap().rearrange("m p -> p m"))

        # row vectors for broadcasts
        rows = {}
        for nm in ["norm1_g", "norm1_b", "ccg", "ccb2", "fusion_b", "ffn_b2"]:
            rows[nm] = rowpool.tile([1, D], F32, tag=nm, name="row_" + nm)
            nc.sync.dma_start(rows[nm][:], di[nm].ap())
        rows["gate_b"] = rowpool.tile([1, 2], F32, tag="gate_b", name="row_gate_b")
        nc.sync.dma_start(rows["gate_b"][:], di["gate_b"].ap())

        # broadcast [1,D] rows across partitions via ones-matmul
        bc = {}
        with tc.tile_pool(name="bcps", bufs=2, space="PSUM") as pps:
            for nm in ["norm1_g", "norm1_b", "ccg", "ccb2", "fusion_b", "ffn_b2", "gate_b"]:
                w = rows[nm].shape[1]
                bct = pk.tile([P, w], F32, tag="bc_" + nm, name="bc_" + nm)
                ps = pps.tile([P, 512], F32, tag="bcps")
                nc.tensor.matmul(ps[:, :w], ones1f32[:], rows[nm][:], start=True, stop=True)
                nc.scalar.copy(bct[:], ps[:, :w])
                bc[nm] = bct

        # main weights (DMAs emitted after phase 1 so x loads first)
        winz_sb = pw.tile([P, 4, 1536], F16)
        nc.gpsimd.dma_start(winz_sb[:], di["wT_inz"].ap().rearrange("(k p) m -> p k m", p=P))
        wxp_sb = pw.tile([P, 8, 64], F16)
        nc.gpsimd.dma_start(wxp_sb[:], di["wT_xproj"].ap().rearrange("(k p) m -> p k m", p=P))
        wdt_sb = pw.tile([DT_RANK, DH], F16)
        nc.gpsimd.dma_start(wdt_sb[:], di["wT_dt"].ap())
        wout_sb = pw.tile([P, 4, D], F16)
        nc.gpsimd.dma_start(wout_sb[:], di["wT_out"].ap().rearrange("(k p) m -> p k m", p=P))

        # ========== Phase 2+3 emitters (per time-half th of 512 tokens) ====
        # th=0 runs inline (scan chunk 0 gates on it); th=1, the z-gate rows,
        # and the token-tail head are deferred as closures popped one per scan
        # iteration, filling PE/Act idle slots under the scan.
        xcT = pw.tile([P, 8, L], F16)       # full-DI conv output (permuted order)
        zT = pw.tile([P, 4, L], F16)        # silu(z) for my half
        delta = pw.tile([P, 4, L], F16)
        dtT = pxn.tile([DT_RANK, L], F16)
        p2c = early.enter_context(tc.tile_pool(name="ph2c", bufs=1))
        p2x = early.enter_context(tc.tile_pool(name="ph2x", bufs=1))
        p2ps = early.enter_context(tc.tile_pool(name="ph2ps", bufs=1, space="PSUM"))
        p3b = early.enter_context(tc.tile_pool(name="ph3b", bufs=1))
        xppA = p2x.tile([P, 3 + L], F16, tag="xppA")
        nc.vector.memset(xppA[:, 0:3], 0.0)
        xppB = p2x.tile([P, 3 + L], F16, tag="xppB")
        nc.vector.memset(xppB[:, 0:3], 0.0)

        def em_inproj(mt, th):
            ps = p2ps.tile([P, 512], F32, tag=f"thps{mt % 3}", name="zps")
            for kd in range(4):
                nc.tensor.matmul(
                    ps[:], winz_sb[:, kd, mt * P:(mt + 1) * P],
                    xnT[:, kd, th * 512:(th + 1) * 512],
                    start=(kd == 0), stop=(kd == 3))
            xpp = xppA if mt % 2 == 0 else xppB
            if th == 0:
                nc.vector.tensor_scalar_add(
                    xpp[:, 3 + th * 512: 3 + (th + 1) * 512], ps[:],
                    biasz_sb[:, mt:mt + 1])
            else:
                nc.scalar.activation(xpp[:, 3 + th * 512: 3 + (th + 1) * 512],
                                     ps[:], AF.Identity,
                                     bias=biasz_sb[:, mt:mt + 1])

        def em_conv(mt, th):
            # depthwise conv on PE: accumulating matmuls with diag(w_k)
            xpp = xppA if mt % 2 == 0 else xppB
            dgw = p2c.tile([P, KCONV, P], F16, tag="dgw")
            for k in range(KCONV):
                nc.vector.tensor_scalar_mul(dgw[:, k, :], idf16[:],
                                            convw_sb[:, mt, k:k + 1])
            cps = p2ps.tile([P, 512], F32, tag=f"thps{mt % 3}", name="cps")
            for k in range(KCONV):
                nc.tensor.matmul(cps[:], dgw[:, k, :],
                                 xpp[:, k + th * 512: k + th * 512 + 512],
                                 start=(k == 0), stop=(k == 3))
            nc.scalar.activation(xcT[:, mt, th * 512:(th + 1) * 512], cps[:],
                                 AF.Silu, bias=convb_sb[:, mt:mt + 1])

        def em_inconv(mt, th):
            em_inproj(mt, th)
            em_conv(mt, th)

        def em_xproj(th):
            ps = p2ps.tile([64, 512], F32, tag="thps0", name="xdps")
            for kd in range(8):
                nc.tensor.matmul(ps[:], wxp_sb[:, kd, :],
                                 xcT[:, kd, th * 512:(th + 1) * 512],
                                 start=(kd == 0), stop=(kd == 7))
            if th == 0:
                nc.vector.tensor_copy(dtT[:, th * 512:(th + 1) * 512],
                                      ps[0:DT_RANK, :])
            else:
                nc.scalar.copy(dtT[:, th * 512:(th + 1) * 512], ps[0:DT_RANK, :])
            bctmp = p3b.tile([32, 512], F16, tag="bctmp")
            if th == 0:
                nc.vector.tensor_copy(bctmp[:], ps[32:64, :])
            else:
                nc.scalar.copy(bctmp[:], ps[32:64, :])
            nc.sync.dma_start(bc_dram.ap()[:, th * 512:(th + 1) * 512], bctmp[:])

        def em_delta(m, th):
            ps = p2ps.tile([P, 512], F32, tag=f"thps{m % 3}", name="dtps")
            nc.tensor.matmul(ps[:], wdt_sb[:, m * P:(m + 1) * P],
                             dtT[:, th * 512:(th + 1) * 512],
                             start=True, stop=True)
            esc = p3b.tile([P, 512], F16, tag="esc")
            nc.scalar.activation(esc[:], ps[:], AF.Exp, bias=dtb_sb[:, m:m + 1])
            nc.scalar.activation(delta[:, m, th * 512:(th + 1) * 512],
                                 esc[:], AF.Ln, bias=1.0)

        def em_z(mt, th):
            ps = p2ps.tile([P, 512], F32, tag=f"thps{mt % 3}", name="zzps")
            for kd in range(4):
                nc.tensor.matmul(
                    ps[:], winz_sb[:, kd, mt * P:(mt + 1) * P],
                    xnT[:, kd, th * 512:(th + 1) * 512],
                    start=(kd == 0), stop=(kd == 3))
            nc.scalar.activation(zT[:, mt - 8, th * 512:(th + 1) * 512], ps[:],
                                 AF.Silu, bias=biasz_sb[:, mt:mt + 1])

        def em_toktail():
            # token-tail head (xn slice + cc/gate weight loads + transposes)
            ctx = tc.tile_wait_until(0.001 * TOKTAIL_WAIT)
            ctx.__enter__()
            xn_sl = layer_norm(xtok, 2, ptt, pttb,
                               gb=(bc["norm1_g"], bc["norm1_b"]),
                               out_dtype=F16, tag="lnsl")
            tt_tiles["xn_sl"] = xn_sl
            xnsT = ptt.tile([P, 4, 256], F16, tag="xnsT", name="xnsT")
            for tt in range(2):
                pst = pttps.tile([P, 4, P], F16, tag="ps6", name="ttps")
                for dd in range(4):
                    nc.tensor.transpose(pst[:, dd, :],
                                        xn_sl[:, tt, dd * P:(dd + 1) * P],
                                        idf16[:])
                nc.vector.tensor_copy(xnsT[:, :, tt * P:(tt + 1) * P], pst[:])
            tt_tiles["xnsT"] = xnsT
            ctx.__exit__(None, None, None)

        # th=0 chain inline: scan chunk 0 can start after this
        for mt in range(8):
            em_inconv(mt, 0)
        em_xproj(0)
        for m in range(4):
            em_delta(m, 0)
        with tc.tile_wait_until(0.040):
            for mt in range(8, 12):
                em_z(mt, 0)

        # deferred th=1 work, popped into the scan loop: one slot per (m, ngi)
        # iteration plus one per m-boundary (20 slots per chunk).  Hard
        # deadlines: conv(mt,1) all before xproj(1); delta(m,1) before chunk-1
        # iterations of m; z(mt,1) before chunk-1 ypost of its m.
        A_ = lambda mt: (lambda: em_inproj(mt, 1))
        B_ = lambda mt: (lambda: em_conv(mt, 1))
        Z_ = lambda mt: (lambda: em_z(mt, 1))
        D_ = lambda m: (lambda: em_delta(m, 1))
        deferred = [
            # ch0-m0 iters + end          # ch0-m1
            A_(0), A_(1), B_(0), A_(2), B_(1),
            A_(3), B_(2), A_(4), B_(3), A_(5),
            # ch0-m2                      # ch0-m3
            B_(4), A_(6), B_(5), A_(7), B_(6),
            B_(7), lambda: em_xproj(1), D_(0), D_(1), D_(2),
            # ch1-m0 iters + end
            D_(3), Z_(8), Z_(9), Z_(10), Z_(11),
            em_toktail,
        ]
        N_SLOTS = len(deferred)

        # token-tail x slice + small weights (DMA only; compute is deferred)
        tt_tiles = {}
        xtok = ptt.tile([P, 2, D], F32, tag="xtok")
        nc.sync.dma_start(xtok[:], di["x_tok"].ap().rearrange("(k p) d -> p k d", p=P))
        cw_sb = ptt.tile([P, 4, D], F16, tag="ccw")
        nc.gpsimd.dma_start(cw_sb[:], di["cc_wT"].ap().rearrange("(k p) m -> p k m", p=P))
        cnT_sb = ptt.tile([P, 4, NC_CLUST], F16, tag="cnT")
        nc.gpsimd.dma_start(cnT_sb[:], di["centers_nT"].ap().rearrange("(k p) m -> p k m", p=P))
        cent_sb = ptt.tile([NC_CLUST, D], F16, tag="cent")
        nc.gpsimd.dma_start(cent_sb[:], di["centers_dev"].ap())
        gw_sb = ptt.tile([P, 4, 2], F16, tag="gw")
        nc.gpsimd.dma_start(gw_sb[:], di["gate_wT"].ap().rearrange("(k p) m -> p k m", p=P))

        # ================= Phase 4+5: scan, y, out_proj ====================
        # n-sum strategy: pprod partials are accumulated over n on the PE
        # (identity-matmul into PSUM, idle during the scan), with D*xc folded
        # in as a diagonal matmul; dBu stays on DVE, pprod mostly on Pool.
        hprev = pw.tile([P, 4, NST], F16)
        dgD = pw.tile([P, 4, P], F16)       # diag(D) per m-block
        for m in range(4):
            nc.vector.tensor_scalar_mul(dgD[:, m, :], idf16[:], Dp_sb[:, m:m + 1])
        with tc.tile_pool(name="ph4", bufs=3) as p4, \
             tc.tile_pool(name="ph4bc", bufs=2) as p4bc, \
             tc.tile_pool(name="ph4da", bufs=2) as p4da, \
             tc.tile_pool(name="ph4y1", bufs=1) as p4y1, \
             tc.tile_pool(name="ph4ps", bufs=2, space="PSUM") as p4ps, \
             tc.tile_pool(name="ph5ps", bufs=1, space="PSUM") as p5ps:
            n_ch = L // TC
            for ch in range(n_ch):
                t0 = ch * TC
                yTf = p4y1.tile([P, 4, TC], F16, tag="yTf", name="yTf")
                dus = p4y1.tile([P, 4, TC], F16, tag="dus", name="dus")
                outT = p4y1.tile([P, 2, 2 * D], F16, tag="outT", name="outT")
                for m in range(4):
                    psy = p4ps.tile([P, TC], F32, tag="psy", name="psy")
                    nc.vector.tensor_mul(dus[:, m, :],
                                         delta[:, m, t0:t0 + TC],
                                         xcT[:, m, t0:t0 + TC])
                    for ngi in range(NST // NG):
                        nbase = ngi * NG
                        Bb = p4bc.tile([P, NG, TC], F16, tag="Bb")
                        nc.sync.dma_start(
                            Bb[:], bc_dram.ap()[None, nbase:nbase + NG, t0:t0 + TC]
                            .to_broadcast((P, NG, TC)))
                        Cb = p4bc.tile([P, NG, TC], F16, tag="Cb")
                        nc.sync.dma_start(
                            Cb[:], bc_dram.ap()[None, 16 + nbase:16 + nbase + NG, t0:t0 + TC]
                            .to_broadcast((P, NG, TC)))
                        dA = p4da.tile([P, NG, TC], F16, tag="dA")
                        for j in range(NG):
                            if a_vals is not None:
                                nc.scalar.activation(
                                    dA[:, j, :], delta[:, m, t0:t0 + TC], AF.Exp,
                                    scale=float(a_vals[nbase + j]))
                            else:
                                nc.scalar.activation(
                                    dA[:, j, :], delta[:, m, t0:t0 + TC], AF.Exp,
                                    scale=A_sb[:, m, nbase + j:nbase + j + 1])
                        if deferred:
                            deferred.pop(0)()
                        dBu = p4.tile([P, NG, TC], F16, tag="dBu")
                        nc.vector.tensor_tensor(
                            dBu[:], dus[:, m, None, :].to_broadcast((P, NG, TC)),
                            Bb[:], AL.mult)
                        h = p4.tile([P, NG, TC], F16, tag="h")
                        for j in range(NG):
                            init = 0.0 if ch == 0 else hprev[:, m, nbase + j:nbase + j + 1]
                            nc.vector.tensor_tensor_scan(
                                h[:, j, :], dA[:, j, :], dBu[:, j, :], init,
                                AL.mult, AL.add)
                        if ch < n_ch - 1:
                            nc.vector.tensor_copy(hprev[:, m, nbase:nbase + NG],
                                                  h[:, :, TC - 1])
                        pprod = p4.tile([P, NG, TC], F16, tag="pprod", name="pprod")
                        # pprod split DVE:Pool to unload DVE (the scan engine)
                        # while keeping the Pool link short
                        if PPROD_DVE_J > 0:
                            nc.vector.tensor_mul(pprod[:, 0:PPROD_DVE_J, :],
                                                 h[:, 0:PPROD_DVE_J, :],
                                                 Cb[:, 0:PPROD_DVE_J, :])
                        if PPROD_DVE_J < NG:
                            nc.gpsimd.tensor_mul(pprod[:, PPROD_DVE_J:, :],
                                                 h[:, PPROD_DVE_J:, :],
                                                 Cb[:, PPROD_DVE_J:, :])
                        # n-sum on PE: psy += sum_j pprod[:, j, :]
                        for j in range(NG):
                            nc.tensor.matmul(psy[:], idf16[:], pprod[:, j, :],
                                             start=(ngi == 0 and j == 0),
                                             stop=False)
                    # finish psum: += diag(D) @ xc, then gate with silu(z) + flip
                    nc.tensor.matmul(psy[:], dgD[:, m, :], xcT[:, m, t0:t0 + TC],
                                     start=False, stop=True)
                    yp_eng = nc.gpsimd if YPOST_POOL else nc.vector
                    if BUILD_NOIF:
                        yp_eng.tensor_tensor(yTf[:, m, :], psy[:],
                                             zT[:, m, t0:t0 + TC], AL.mult)
                    else:
                        pid = nc.partition_id()
                        with tc.If(pid >= 4) as cmp:
                            yp_eng.tensor_tensor(
                                yTf[:, m, :], psy[:, ::-1],
                                zT[:, m, t0:t0 + TC][:, ::-1], AL.mult)
                        with cmp.Else():
                            yp_eng.tensor_tensor(yTf[:, m, :], psy[:],
                                                 zT[:, m, t0:t0 + TC], AL.mult)
                    if deferred:
                        slot = N_SLOTS - len(deferred)
                        with tc.tile_wait_until(0.001 * (28 + 3.6 * slot)):
                            deferred.pop(0)()
                # out_proj (token-part output); for backward cores this chunk's
                # yTf holds true tokens [L-t0-TC, L-t0), i.e. chunk (n_ch-1-ch)
                for tt in range(4):
                    ps = p5ps.tile([P, 512], F32, tag="ops")
                    for m in range(4):
                        nc.tensor.matmul(ps[:], yTf[:, m, tt * P:(tt + 1) * P],
                                         wout_sb[:, m, :],
                                         start=(m == 0), stop=(m == 3))
                    nc.scalar.copy(outT[:, tt // 2, (tt % 2) * D:(tt % 2 + 1) * D], ps[:])
                pchs = [2 * ch, 2 * ch + 1]
                if BUILD_NOIF:
                    for p_ch in pchs:
                        for sub in range(2):
                            nc.sync.dma_start(
                                rs_in.ap()[p_ch, sub * P:(sub + 1) * P, :],
                                outT[:, p_ch - 2 * ch, sub * D:(sub + 1) * D])
                else:
                    with tc.If(pid >= 4) as cmp2:
                        for p_ch in pchs:
                            for sub in range(2):
                                nc.sync.dma_start(
                                    rs_in.ap()[p_ch ^ 2, sub * P:(sub + 1) * P, :],
                                    outT[:, p_ch - 2 * ch, sub * D:(sub + 1) * D])
                    with cmp2.Else():
                        for p_ch in pchs:
                            for sub in range(2):
                                nc.sync.dma_start(
                                    rs_in.ap()[p_ch, sub * P:(sub + 1) * P, :],
                                    outT[:, p_ch - 2 * ch, sub * D:(sub + 1) * D])

        early.close()

        # ====== Token-tail part 2: cc path, gate ====
        # (virtual release time keeps the greedy scheduler from hoisting these
        # Act/PE ops ahead of the scan-critical head chain)
        part2 = ExitStack()
        part2.enter_context(tc.tile_wait_until(0.001 * PART2_WAIT))
        xn_sl = tt_tiles["xn_sl"]
        xnsT = tt_tiles["xnsT"]
        projT = ptt.tile([P, 4, 256], F16, tag="projT")
        sqT = ptt.tile([P, 4, 256], F16, tag="sqT")
        for pf in range(4):
            ps = pttps.tile([P, 256], F32, tag="ps6")
            for kd in range(4):
                nc.tensor.matmul(ps[:], cw_sb[:, kd, pf * P:(pf + 1) * P],
                                 xnsT[:, kd, :], start=(kd == 0), stop=(kd == 3))
            nc.scalar.activation(projT[:, pf, :], ps[:], AF.Identity,
                                 bias=ccbias_sb[:, pf:pf + 1])
            nc.scalar.activation(sqT[:, pf, :], projT[:, pf, :], AF.Square)
        onescol = ptt.tile([P, 1], F16, tag="onescol")
        nc.vector.memset(onescol[:], 1.0)
        stack = ptt.tile([16, 256], F32, tag="stack")
        nc.vector.memset(stack[:], 0.0)
        ps_sim = pttps.tile([NC_CLUST, 256], F32, tag="pst6", name="ps_sim")
        for kd in range(4):
            nc.tensor.matmul(ps_sim[:], cnT_sb[:, kd, :], projT[:, kd, :],
                             start=(kd == 0), stop=(kd == 3))
        nc.scalar.copy(stack[0:8, :], ps_sim[:])
        ps_ssq = pttps.tile([1, 256], F32, tag="pst6", name="ps_ssq")
        for kd in range(4):
            nc.tensor.matmul(ps_ssq[:], onescol[:], sqT[:, kd, :],
                             start=(kd == 0), stop=(kd == 3))
        ssq_tmp = ptt.tile([1, 256], F32, tag="ssq_tmp")
        nc.scalar.copy(ssq_tmp[:], ps_ssq[:])
        nc.sync.dma_start(stack[8:9, :], ssq_tmp[:])
        S = ptt.tile([P, 2, 16], F32, tag="S")
        for tt in range(2):
            pst = pttps.tile([P, 16], F32, tag="pst6", name="stps")
            nc.tensor.transpose(pst[:], stack[:, tt * P:(tt + 1) * P],
                                idf32[:])
            nc.scalar.copy(S[:, tt, :], pst[:])
        nrm = ptt.tile([P, 2], F32, tag="nrm")
        nc.scalar.sqrt(nrm[:], S[:, :, 8])
        nc.vector.tensor_scalar_max(nrm[:], nrm[:], 1e-12)
        rnrm = ptt.tile([P, 2], F32, tag="rnrm")
        nc.vector.reciprocal(rnrm[:], nrm[:])
        wcl = ptt.tile([P, 2, NC_CLUST], F16, tag="wcl")
        for tt in range(2):
            sim = pttb.tile([P, NC_CLUST], F32, tag="sim")
            nc.vector.tensor_scalar_mul(sim[:], S[:, tt, 0:8], rnrm[:, tt:tt + 1])
            mx = pttb.tile([P, 1], F32, tag="mx")
            nc.vector.tensor_reduce(mx[:], sim[:], AX.X, AL.max)
            nmx = pttb.tile([P, 1], F32, tag="nmx")
            nc.vector.tensor_scalar_mul(nmx[:], mx[:], -1.0)
            se = pttb.tile([P, 1], F32, tag="se")
            ex = pttb.tile([P, NC_CLUST], F32, tag="ex")
            nc.scalar.activation(ex[:], sim[:], AF.Exp, bias=nmx[:], accum_out=se[:])
            rse = pttb.tile([P, 1], F32, tag="rse")
            nc.vector.reciprocal(rse[:], se[:])
            nc.vector.tensor_scalar_mul(wcl[:, tt, :], ex[:], rse[:])
        wclT = ptt.tile([NC_CLUST, 256], F16, tag="wclT")
        for tt in range(2):
            pst = pttps.tile([NC_CLUST, P], F16, tag="pst6", name="wtps")
            nc.tensor.transpose(pst[:], wcl[:, tt, :], idf16[:])
            nc.scalar.copy(wclT[:, tt * P:(tt + 1) * P], pst[:])
        ccpre = ptt.tile([P, 2, D], F32, tag="ccpre")
        for tt in range(2):
            ps = pttps.tile([P, D], F32, tag="ps6", name="ctxps")
            nc.tensor.matmul(ps[:], wclT[:, tt * P:(tt + 1) * P], cent_sb[:],
                             start=True, stop=True)
            nc.vector.scalar_tensor_tensor(ccpre[:, tt, :], ps[:], alpha_sb[:],
                                           xn_sl[:, tt, :], AL.mult, AL.add)
        cc_out = layer_norm(ccpre, 2, ptt, pttb, gb=(bc["ccg"], bc["ccb2"]),
                            out_dtype=F32, tag="lncc")

        gcl = ptt.tile([P, 2, 2], F32, tag="gcl")
        for tt in range(2):
            ps = pttps.tile([P, D], F32, tag="ps6", name="gps")
            for kd in range(4):
                nc.tensor.matmul(ps[:, 0:2], xnsT[:, kd, tt * P:(tt + 1) * P],
                                 gw_sb[:, kd, :], start=(kd == 0), stop=(kd == 3))
            gpre = pttb.tile([P, 2], F32, tag="gpre")
            nc.vector.tensor_add(gpre[:], ps[:, 0:2], bc["gate_b"][:])
            mx = pttb.tile([P, 1], F32, tag="gmx")
            nc.vector.tensor_reduce(mx[:], gpre[:], AX.X, AL.max)
            nmx = pttb.tile([P, 1], F32, tag="gnmx")
            nc.vector.tensor_scalar_mul(nmx[:], mx[:], -1.0)
            se = pttb.tile([P, 1], F32, tag="gse")
            ex = pttb.tile([P, 2], F32, tag="gex")
            nc.scalar.activation(ex[:], gpre[:], AF.Exp, bias=nmx[:], accum_out=se[:])
            rse = pttb.tile([P, 1], F32, tag="grse")
            nc.vector.reciprocal(rse[:], se[:])
            nc.vector.tensor_scalar_mul(gcl[:, tt, :], ex[:], rse[:])

        part2.close()
        if BUILD_NOCC:
            nc.sync.dma_start(rs_out.ap(), rs_in.ap()[0])
        else:
            nc.gpsimd.collective_compute(
                "ReduceScatter", AL.add, ins=[rs_in.ap()], outs=[rs_out.ap()],
                replica_groups=RG)
        mid.close()

        # ================= Late tail: fuse + FFN ===========================
        with tc.tile_pool(name="ph6", bufs=1) as p6, \
             tc.tile_pool(name="ph6b", bufs=2) as p6b, \
             tc.tile_pool(name="ph6ps", bufs=2, space="PSUM") as p6ps:
            mamba16 = p6.tile([P, 2, D], F16, tag="mamba16")
            nc.sync.dma_start(mamba16[:], rs_out.ap().rearrange("(k p) d -> p k d", p=P))
            mamba = p6.tile([P, 2, D], F32, tag="mamba")
            nc.vector.tensor_tensor(
                mamba[:], mamba16[:],
                bc["fusion_b"][:, None, :].to_broadcast((P, 2, D)), AL.add)

            # t0c precomputed (gcl/cc_out ready before the collective lands)
            t0c = p6.tile([P, 2, D], F32, tag="t0c")
            for tt in range(2):
                nc.vector.tensor_scalar_mul(t0c[:, tt, :], cc_out[:, tt, :],
                                            gcl[:, tt, 1:2])
                nc.vector.tensor_add(t0c[:, tt, :], t0c[:, tt, :], xtok[:, tt, :])
            x2 = p6.tile([P, 2, D], F32, tag="x2")
            for tt in range(2):
                nc.vector.scalar_tensor_tensor(x2[:, tt, :], mamba[:, tt, :],
                                               gcl[:, tt, 0:1], t0c[:, tt, :],
                                               AL.mult, AL.add)

            hln = layer_norm(x2, 2, p6, p6b, gb=None, out_dtype=F16, tag="lnffn")
            hT = p6.tile([P, 4, 256], F16, tag="hT")
            for tt in range(2):
                pst = p6ps.tile([P, 4, P], F16, tag="ps6", name="htps")
                for dd in range(4):
                    nc.tensor.transpose(pst[:, dd, :],
                                        hln[:, tt, dd * P:(dd + 1) * P],
                                        idf16[:])
                nc.vector.tensor_copy(hT[:, :, tt * P:(tt + 1) * P], pst[:])
            w1_sb = p6.tile([P, 4, 4 * D], F16, tag="w1")
            nc.gpsimd.dma_start(w1_sb[:], di["ffn_w1T"].ap().rearrange("(k p) m -> p k m", p=P))
            w2_sb = p6.tile([P, 16, D], F16, tag="w2")
            nc.gpsimd.dma_start(w2_sb[:], di["ffn_w2T"].ap().rearrange("(k p) m -> p k m", p=P))
            gT = p6.tile([P, 16, 256], F16, tag="gT")
            for gq in range(4):
                ps = p6ps.tile([P, 2, 256], F32, tag="ps6", name="f1ps")
                for gh in range(2):
                    gf = 2 * gq + gh
                    for kd in range(4):
                        nc.tensor.matmul(ps[:, gh, :],
                                         w1_sb[:, kd, gf * P:(gf + 1) * P],
                                         hT[:, kd, :], start=(kd == 0), stop=(kd == 3))
                nc.scalar.activation(gT[:, 2 * gq, :], ps[:, 0, :], AF.Gelu,
                                     bias=ffnb1_sb[:, 2 * gq:2 * gq + 1])
                nc.scalar.activation(gT[:, 2 * gq + 1, :], ps[:, 1, :], AF.Gelu,
                                     bias=ffnb1_sb[:, 2 * gq + 1:2 * gq + 2])
            for gq in range(4, 8):
                ps = p6ps.tile([P, 2, 256], F32, tag="ps6", name="f1ps")
                for gh in range(2):
                    gf = 2 * gq + gh
                    for kd in range(4):
                        nc.tensor.matmul(ps[:, gh, :],
                                         w1_sb[:, kd, gf * P:(gf + 1) * P],
                                         hT[:, kd, :], start=(kd == 0), stop=(kd == 3))
                nc.scalar.activation(gT[:, 2 * gq, :], ps[:, 0, :], AF.Gelu,
                                     bias=ffnb1_sb[:, 2 * gq:2 * gq + 1])
                nc.scalar.activation(gT[:, 2 * gq + 1, :], ps[:, 1, :], AF.Gelu,
                                     bias=ffnb1_sb[:, 2 * gq + 1:2 * gq + 2])
            for tt in range(2):
                ps = p6ps.tile([P, D], F32, tag="ps6", name="f2ps")
                for gf in range(16):
                    nc.tensor.matmul(ps[:], gT[:, gf, tt * P:(tt + 1) * P],
                                     w2_sb[:, gf, :], start=(gf == 0), stop=(gf == 15))
                ot = p6b.tile([P, D], F32, tag="ot")
                nc.vector.tensor_add(ot[:], ps[:], x2[:, tt, :])
                nc.vector.tensor_add(ot[:], ot[:], bc["ffn_b2"][:])
                nc.sync.dma_start(
                    out_slice.ap().rearrange("(k p) d -> p k d", p=P)[:, tt, :], ot[:])

    return nc


def _prep_inputs(inputs):
    """Build the 8 per-core input dicts from the full problem inputs."""
    x = _f32(inputs["x"])
    in_maps = []
    for c in range(N_CORES):
        half = c & 1
        batch = (c >> 1) & 1
        flip = c >= 4
        pos = (c & 1) + 2 * (c >> 2)
        pfx = "bm_" if flip else "fm_"
        g = lambda k: np.asarray(inputs[pfx + k])

        perm = np.r_[half * DH:(half + 1) * DH, (1 - half) * DH:(2 - half) * DH]
        in_w = np.asarray(g("in_w"))          # [2048, 512]
        xp_w = in_w[:DI][perm]
        z_w = in_w[DI + half * DH: DI + (half + 1) * DH]
        W_inz = np.concatenate([xp_w, z_w], axis=0)         # [1536, 512]
        n1g = _f32(inputs["norm1_g"])
        n1b = _f32(inputs["norm1_b"])
        wT_inz = _dt((W_inz * n1g[None, :]).T)
        bias_inz = _f32(W_inz @ n1b).reshape(12, P)

        xproj_w = np.asarray(g("xproj_w"))                  # [64, 1024]
        wT_xproj = _dt(xproj_w[:, perm].T)

        dt_w = np.asarray(g("dt_w"))                        # [1024, 32]
        wT_dt = _dt(dt_w[half * DH:(half + 1) * DH].T)
        dt_bias = _f32(g("dt_b")[half * DH:(half + 1) * DH]).reshape(4, P)

        A = -np.exp(_f32(g("A_log")))
        A_dev = _f32(A[half * DH:(half + 1) * DH])

        convw = _f32(g("conv_w")[:, 0, :][perm])
        convb = _f32(g("conv_b")[perm]).reshape(8, P)
        Dp_dev = _f32(g("D")[half * DH:(half + 1) * DH]).reshape(4, P)

        fusion_w = np.asarray(inputs["fusion_w"])
        # fusion input is concat(f_out, b_out): f -> cols 0:512, b -> 512:1024
        Wdir = fusion_w[:, 512:1024] if flip else fusion_w[:, 0:512]
        M = Wdir @ np.asarray(g("out_w"))                   # [512o, 1024di]
        wT_out = _dt(M[:, half * DH:(half + 1) * DH].T)

        centers = _f32(inputs["cc_centers"])
        cn = centers / np.maximum(np.linalg.norm(centers, axis=-1, keepdims=True), 1e-12)

        d = {
            "x_full": _f32(x[batch, ::-1] if flip else x[batch]),
            "x_tok": _f32(x[batch, pos * 256:(pos + 1) * 256]),
            "wT_inz": wT_inz,
            "bias_inz": bias_inz,
            "wT_xproj": wT_xproj,
            "wT_dt": wT_dt,
            "dt_bias": dt_bias,
            "A_dev": A_dev,
            "convw": convw,
            "convb": convb,
            "Dp_dev": Dp_dev,
            "wT_out": wT_out,
            "fusion_b": _f32(inputs["fusion_b"]).reshape(1, D),
            "cc_wT": _dt(np.asarray(inputs["cc_proj_w"]).T),
            "ccb": _f32(inputs["cc_proj_b"]).reshape(4, P),
            "centers_nT": _dt(cn.T),
            "centers_dev": _dt(centers),
            "norm1_g": n1g.reshape(1, D),
            "norm1_b": n1b.reshape(1, D),
            "ccg": _f32(inputs["cc_norm_g"]).reshape(1, D),
            "ccb2": _f32(inputs["cc_norm_b"]).reshape(1, D),
            "alpha_col": np.full((P, 1), float(np.asarray(inputs["cc_alpha"]).ravel()[0]), np.float32),
            "gate_wT": _dt(np.asarray(inputs["gate_w"]).T),
            "gate_b": _f32(inputs["gate_b"]).reshape(1, 2),
            "ffn_w1T": _dt((np.asarray(inputs["ffn_w1"]) * _f32(inputs["ffn_norm_g"])[None, :]).T),
            "ffn_b1": _f32(np.asarray(inputs["ffn_b1"]) + np.asarray(inputs["ffn_w1"]) @ _f32(inputs["ffn_norm_b"])).reshape(16, P),
            "ffn_w2T": _dt(np.asarray(inputs["ffn_w2"]).T),
            "ffn_b2": _f32(inputs["ffn_b2"]).reshape(1, D),
        }
        in_maps.append(d)
    return in_maps


TRACE = False
LAST_RESULT = {}


def _detect_uniform_A(inputs):
    As = [-np.exp(_f32(np.asarray(inputs[p + "A_log"]))) for p in ("fm_", "bm_")]
    a0 = As[0][0]
    for A in As:
        if not np.allclose(A, a0[None, :], rtol=0, atol=0):
            return None
    return tuple(float(v) for v in a0)


def kernel(**inputs):
    a_vals = _detect_uniform_A(inputs)
    key = ("nc", a_vals)
    if key not in _CACHED:
        nc = _build_nc(a_vals=a_vals)
        split_multi_waits(nc)
        _CACHED[key] = nc
    nc = _CACHED[key]
    in_maps = _prep_inputs(inputs)
    res = run_bass_kernel_spmd(nc, in_maps, core_ids=list(range(N_CORES)),
                               trace=TRACE)
    LAST_RESULT["res"] = res
    out = np.empty((2, L, D), np.float32)
    for c in range(N_CORES):
        batch = (c >> 1) & 1
        pos = (c & 1) + 2 * (c >> 2)
        out[batch, pos * 256:(pos + 1) * 256] = res.results[c]["out_slice"]
    return out



# revision 53
# speedup vs baseline: 1.1102x; 1.0124x over previous
"""CCBiMambaBlock fused kernel for 8 trn2 NeuronCores.

Sharding: 8 cores = (batch 2) x (direction 2) x (DI-half 2), SPMD (one
program, per-core data). Backward-direction cores receive host-flipped x.
Core map: 0,1 = b0 fwd halves; 2,3 = b1 fwd; 4,5 = b0 bwd; 6,7 = b1 bwd.
The fusion matmul is host-folded into out_proj (M = fusion_w_dir @ out_w), so
mamba_out = sum over (dir, half) of partial projections -> one ReduceScatter
per 4-core batch group, sharding tokens 4-way for the token-parallel tail
(context-clustering, gate, FFN). The token-tail's collective-independent part
(cc path, gate) is emitted early so it fills scan-phase engine idle slots.
"""
import numpy as np
from contextlib import ExitStack

import concourse.bass as bass
import concourse.mybir as mybir
import concourse.tile as tile
from concourse.bass_utils import run_bass_kernel_spmd
from concourse.masks import make_identity

F32 = mybir.dt.float32
F16 = mybir.dt.float16
AL = mybir.AluOpType
AF = mybir.ActivationFunctionType
AX = mybir.AxisListType

P = 128
L = 1024          # tokens per batch
D = 512           # d_model
DI = 1024         # d_inner
DH = 512          # DI per core (half)
NST = 16          # d_state
DT_RANK = 32
KCONV = 4
NC_CLUST = 8
TC = 512          # scan time-chunk
NG = 4            # states per n-group
EPS = 1e-5
N_CORES = 8

_CACHED = {}
BUILD_NOIF = False  # timing builds: emit fwd branch only (TimelineSim can't branch)
BUILD_NOCC = False  # timing builds: replace collective with local DMA copy

# pprod n-groups 0..PPROD_DVE_J-1 run on DVE, the rest on Pool, so the DVE
# (which owns the scans) and Pool finish the scan phase together.
PPROD_DVE_J = 2


def _dt(x):
    return np.ascontiguousarray(x, dtype=np.float16)


def _f32(x):
    return np.ascontiguousarray(x, dtype=np.float32)


def split_multi_waits(nc, max_waits=1):
    """This walrus build rejects >1 sync waits per instruction; move excess
    waits onto preceding same-engine NoOps."""
    n = 0
    for fn in nc.m.functions:
        for blk in fn.blocks:
            out = []
            for inst in blk.instructions:
                si = inst.sync_info
                if si is not None and si.on_wait and len(si.on_wait) > max_waits:
                    waits = list(si.on_wait)
                    excess, keep = waits[:-max_waits], waits[-max_waits:]
                    for i, w in enumerate(excess):
                        out.append(mybir.InstNoOp(
                            name=f"{inst.name}-ws{i}", engine=inst.engine,
                            ins=[], outs=[],
                            sync_info=mybir.SyncInfo(on_wait=[w], on_update=[])))
                        n += 1
                    inst.sync_info = mybir.SyncInfo(
                        on_wait=keep, on_update=list(si.on_update))
                out.append(inst)
            blk.instructions = out
    return n


def _build_nc(a_vals=None):
    nc = bass.Bass("TRN2", target_bir_lowering=False, debug=False,
                   num_devices=N_CORES)

    # ---------------- DRAM I/O ----------------
    di = {}

    def inp(name, shape, dtype):
        di[name] = nc.dram_tensor(name, list(shape), dtype, kind="ExternalInput")
        return di[name]

    inp("x_full", (L, D), F32)
    inp("x_tok", (L // 4, D), F32)
    inp("wT_inz", (D, 1536), F16)
    inp("bias_inz", (12, P), F32)
    inp("wT_xproj", (DI, 64), F16)
    inp("wT_dt", (DT_RANK, DH), F16)
    inp("dt_bias", (4, P), F32)
    inp("A_dev", (DH, NST), F32)
    inp("convw", (DI, KCONV), F32)
    inp("convb", (8, P), F32)
    inp("Dp_dev", (4, P), F32)
    inp("wT_out", (DH, D), F16)
    inp("fusion_b", (1, D), F32)
    inp("cc_wT", (D, D), F16)
    inp("ccb", (4, P), F32)
    inp("centers_nT", (D, NC_CLUST), F16)
    inp("centers_dev", (NC_CLUST, D), F16)
    inp("norm1_g", (1, D), F32)
    inp("norm1_b", (1, D), F32)
    inp("ccg", (1, D), F32)
    inp("ccb2", (1, D), F32)
    inp("alpha_col", (P, 1), F32)
    inp("gate_wT", (D, 2), F16)
    inp("gate_b", (1, 2), F32)
    inp("ffn_w1T", (D, 4 * D), F16)
    inp("ffn_b1", (16, P), F32)
    inp("ffn_w2T", (4 * D, D), F16)
    inp("ffn_b2", (1, D), F32)

    out_slice = nc.dram_tensor("out_slice", [L // 4, D], F32, kind="ExternalOutput")

    rs_in = nc.dram_tensor("rs_in", [4, 256, D], F16)
    rs_out = nc.dram_tensor("rs_out", [256, D], F16)
    bc_dram = nc.dram_tensor("bc_dram", [32, L], F16)   # B rows 0:16, C rows 16:32

    RG = [[0, 1, 4, 5], [2, 3, 6, 7]]

    with tile.TileContext(nc) as tc, ExitStack() as top:
        # persistent pools; `mid` closes before the late tail to free SBUF
        mid = top.enter_context(ExitStack())
        pk = top.enter_context(tc.tile_pool(name="keep", bufs=1))

        rowpool = top.enter_context(tc.tile_pool(name="rows", bufs=1))
        ones1f32 = pk.tile([1, P], F32)
        nc.vector.memset(ones1f32[:], 1.0)
        idf16 = pk.tile([P, P], F16)
        make_identity(nc, idf16[:])
        idf32 = pk.tile([16, 16], F32)
        make_identity(nc, idf32[:])

        # token-tail pools (live to the end)
        ptt = top.enter_context(tc.tile_pool(name="ptt", bufs=1))
        pttb = top.enter_context(tc.tile_pool(name="pttb", bufs=2))
        pttps = top.enter_context(tc.tile_pool(name="pttps", bufs=1, space="PSUM"))

        def layer_norm(src, n_tt, pool, poolb, gb=None, out_dtype=F16, tag="ln"):
            """src [P, n_tt, D] -> normalized tile (optionally * g + b)."""
            st6 = poolb.tile([P, n_tt, 6], F32, tag=tag + "_st6", name=tag + "_st6")
            agg = pool.tile([P, n_tt, 2], F32, tag=tag + "_agg", name=tag + "_agg")
            for tt in range(n_tt):
                nc.vector.bn_stats(st6[:, tt, :], src[:, tt, :])
                nc.vector.bn_aggr(agg[:, tt, :], st6[:, tt, :])
            vr = pool.tile([P, n_tt], F32, tag=tag + "_vr", name=tag + "_vr")
            nc.vector.tensor_scalar_add(vr[:], agg[:, :, 1], EPS)
            nc.scalar.sqrt(vr[:], vr[:])
            rs = pool.tile([P, n_tt], F32, tag=tag + "_rs", name=tag + "_rs")
            nc.vector.reciprocal(rs[:], vr[:])
            o = pool.tile([P, n_tt, D], out_dtype, tag=tag + "_o", name=tag + "_o")
            for tt in range(n_tt):
                nc.vector.tensor_scalar(o[:, tt, :], src[:, tt, :],
                                        agg[:, tt, 0:1], rs[:, tt:tt + 1],
                                        AL.subtract, AL.mult)
                if gb is not None:
                    g_bc, b_bc = gb
                    nc.vector.tensor_mul(o[:, tt, :], o[:, tt, :], g_bc[:])
                    nc.vector.tensor_add(o[:, tt, :], o[:, tt, :], b_bc[:])
            return o

        # ================= Phase 1: LN(x) -> xn, transpose =================
        pw = mid.enter_context(tc.tile_pool(name="mid", bufs=1))
        early = ExitStack()
        pxn = early.enter_context(tc.tile_pool(name="pxn", bufs=1))
        xnT = pxn.tile([P, 4, L], F16)      # [d-part, dblk, t]
        with tc.tile_pool(name="ph1", bufs=2) as p1, \
             tc.tile_pool(name="ph1s", bufs=1) as p1s, \
             tc.tile_pool(name="ph1ps", bufs=2, space="PSUM") as p1ps:
            xsb = p1s.tile([P, 8, D], F32, tag="xsb")
            xr = di["x_full"].ap().rearrange("(k p) d -> p k d", p=P)
            for tt in range(8):
                nc.sync.dma_start(xsb[:, tt, :], xr[:, tt, :])
            st6 = p1s.tile([P, 8, 6], F32, tag="st6")
            agg = p1s.tile([P, 8, 2], F32, tag="agg")
            for tt in range(8):
                nc.vector.bn_stats(st6[:, tt, :], xsb[:, tt, :])
                nc.vector.bn_aggr(agg[:, tt, :], st6[:, tt, :])
            var = p1s.tile([P, 8], F32, tag="var")
            nc.vector.tensor_scalar_add(var[:], agg[:, :, 1], EPS)
            nc.scalar.sqrt(var[:], var[:])
            rstd = p1s.tile([P, 8], F32, tag="rstd")
            nc.vector.reciprocal(rstd[:], var[:])
            xn_tok = p1s.tile([P, 8, D], F16, tag="xntok")
            for tt in range(8):
                nc.vector.tensor_scalar(
                    xn_tok[:, tt, :], xsb[:, tt, :],
                    agg[:, tt, 0:1], rstd[:, tt:tt + 1], AL.subtract, AL.mult)
            # transpose on PE (HWDGE transposes pay ~650ns fixed cost each)
            for tt in range(8):
                pst = p1ps.tile([P, 4, P], F16, tag="tps", name="tps")
                for dd in range(4):
                    nc.tensor.transpose(pst[:, dd, :],
                                        xn_tok[:, tt, dd * P:(dd + 1) * P],
                                        idf16[:])
                nc.vector.tensor_copy(xnT[:, :, tt * P:(tt + 1) * P], pst[:])

        # small per-partition params
        dtb_sb = pk.tile([P, 4], F32)
        nc.gpsimd.dma_start(dtb_sb[:], di["dt_bias"].ap().rearrange("m p -> p m"))
        if a_vals is None:
            A_sb = pk.tile([P, 4, NST], F32)
            nc.sync.dma_start(A_sb[:], di["A_dev"].ap().rearrange("(k p) n -> p k n", p=P))
        convw_sb = pk.tile([P, 8, KCONV], F32)
        nc.gpsimd.dma_start(convw_sb[:], di["convw"].ap().rearrange("(k p) t -> p k t", p=P))
        convb_sb = pk.tile([P, 8], F32)
        nc.gpsimd.dma_start(convb_sb[:], di["convb"].ap().rearrange("k p -> p k"))
        Dp_sb = pk.tile([P, 4], F32)
        nc.gpsimd.dma_start(Dp_sb[:], di["Dp_dev"].ap().rearrange("k p -> p k"))
        alpha_sb = pk.tile([P, 1], F32)
        nc.gpsimd.dma_start(alpha_sb[:], di["alpha_col"].ap())
        biasz_sb = pk.tile([P, 12], F32)
        nc.gpsimd.dma_start(biasz_sb[:], di["bias_inz"].ap().rearrange("m p -> p m"))
        ffnb1_sb = pk.tile([P, 16], F32)
        nc.gpsimd.dma_start(ffnb1_sb[:], di["ffn_b1"].ap().rearrange("m p -> p m"))
        ccbias_sb = pk.tile([P, 4], F32)
        nc.gpsimd.dma_start(ccbias_sb[:], di["ccb"].ap().rearrange("m p -> p m"))

        # row vectors for broadcasts
        rows = {}
        for nm in ["norm1_g", "norm1_b", "ccg", "ccb2", "fusion_b", "ffn_b2"]:
            rows[nm] = rowpool.tile([1, D], F32, tag=nm, name="row_" + nm)
            nc.gpsimd.dma_start(rows[nm][:], di[nm].ap())
        rows["gate_b"] = rowpool.tile([1, 2], F32, tag="gate_b", name="row_gate_b")
        nc.gpsimd.dma_start(rows["gate_b"][:], di["gate_b"].ap())

        # broadcast [1,D] rows across partitions via ones-matmul
        bc = {}
        with tc.tile_pool(name="bcps", bufs=2, space="PSUM") as pps:
            for nm in ["norm1_g", "norm1_b", "ccg", "ccb2", "fusion_b", "ffn_b2", "gate_b"]:
                w = rows[nm].shape[1]
                bct = pk.tile([P, w], F32, tag="bc_" + nm, name="bc_" + nm)
                ps = pps.tile([P, 512], F32, tag="bcps")
                nc.tensor.matmul(ps[:, :w], ones1f32[:], rows[nm][:], start=True, stop=True)
                nc.scalar.copy(bct[:], ps[:, :w])
                bc[nm] = bct

        # main weights (DMAs emitted after phase 1 so x loads first)
        winz_sb = pw.tile([P, 4, 1536], F16)
        nc.gpsimd.dma_start(winz_sb[:], di["wT_inz"].ap().rearrange("(k p) m -> p k m", p=P))
        wxp_sb = pw.tile([P, 8, 64], F16)
        nc.gpsimd.dma_start(wxp_sb[:], di["wT_xproj"].ap().rearrange("(k p) m -> p k m", p=P))
        wdt_sb = pw.tile([DT_RANK, DH], F16)
        nc.gpsimd.dma_start(wdt_sb[:], di["wT_dt"].ap())
        wout_sb = pw.tile([P, 4, D], F16)
        nc.gpsimd.dma_start(wout_sb[:], di["wT_out"].ap().rearrange("(k p) m -> p k m", p=P))

        # ========== Phase 2+3 emitters (per time-half th of 512 tokens) ====
        # th=0 runs inline (scan chunk 0 gates on it); th=1, the z-gate rows,
        # and the token-tail head are deferred as closures popped one per scan
        # iteration, filling PE/Act idle slots under the scan.
        xcT = pw.tile([P, 8, L], F16)       # full-DI conv output (permuted order)
        zT = pw.tile([P, 4, L], F16)        # silu(z) for my half
        delta = pw.tile([P, 4, L], F16)
        dtT = pxn.tile([DT_RANK, L], F16)
        p2c = early.enter_context(tc.tile_pool(name="ph2c", bufs=1))
        p2x = early.enter_context(tc.tile_pool(name="ph2x", bufs=1))
        p2ps = early.enter_context(tc.tile_pool(name="ph2ps", bufs=1, space="PSUM"))
        p3b = early.enter_context(tc.tile_pool(name="ph3b", bufs=1))
        xppA = p2x.tile([P, 3 + L], F16, tag="xppA")
        nc.vector.memset(xppA[:, 0:3], 0.0)
        xppB = p2x.tile([P, 3 + L], F16, tag="xppB")
        nc.vector.memset(xppB[:, 0:3], 0.0)

        def em_inproj(mt, th):
            ps = p2ps.tile([P, 512], F32, tag=f"thps{mt % 3}", name="zps")
            for kd in range(4):
                nc.tensor.matmul(
                    ps[:], winz_sb[:, kd, mt * P:(mt + 1) * P],
                    xnT[:, kd, th * 512:(th + 1) * 512],
                    start=(kd == 0), stop=(kd == 3))
            xpp = xppA if mt % 2 == 0 else xppB
            if th == 0:
                nc.vector.tensor_scalar_add(
                    xpp[:, 3 + th * 512: 3 + (th + 1) * 512], ps[:],
                    biasz_sb[:, mt:mt + 1])
            else:
                nc.scalar.activation(xpp[:, 3 + th * 512: 3 + (th + 1) * 512],
                                     ps[:], AF.Identity,
                                     bias=biasz_sb[:, mt:mt + 1])

        def em_conv(mt, th):
            # depthwise conv on PE: accumulating matmuls with diag(w_k)
            xpp = xppA if mt % 2 == 0 else xppB
            dgw = p2c.tile([P, KCONV, P], F16, tag="dgw")
            for k in range(KCONV):
                nc.vector.tensor_scalar_mul(dgw[:, k, :], idf16[:],
                                            convw_sb[:, mt, k:k + 1])
            cps = p2ps.tile([P, 512], F32, tag=f"thps{mt % 3}", name="cps")
            for k in range(KCONV):
                nc.tensor.matmul(cps[:], dgw[:, k, :],
                                 xpp[:, k + th * 512: k + th * 512 + 512],
                                 start=(k == 0), stop=(k == 3))
            nc.scalar.activation(xcT[:, mt, th * 512:(th + 1) * 512], cps[:],
                                 AF.Silu, bias=convb_sb[:, mt:mt + 1])

        def em_inconv(mt, th):
            em_inproj(mt, th)
            em_conv(mt, th)

        def em_xproj(th):
            ps = p2ps.tile([64, 512], F32, tag="thps0", name="xdps")
            for kd in range(8):
                nc.tensor.matmul(ps[:], wxp_sb[:, kd, :],
                                 xcT[:, kd, th * 512:(th + 1) * 512],
                                 start=(kd == 0), stop=(kd == 7))
            if th == 0:
                nc.vector.tensor_copy(dtT[:, th * 512:(th + 1) * 512],
                                      ps[0:DT_RANK, :])
            else:
                nc.scalar.copy(dtT[:, th * 512:(th + 1) * 512], ps[0:DT_RANK, :])
            bctmp = p3b.tile([32, 512], F16, tag="bctmp")
            if th == 0:
                nc.vector.tensor_copy(bctmp[:], ps[32:64, :])
            else:
                nc.scalar.copy(bctmp[:], ps[32:64, :])
            nc.sync.dma_start(bc_dram.ap()[:, th * 512:(th + 1) * 512], bctmp[:])

        def em_delta(m, th):
            ps = p2ps.tile([P, 512], F32, tag=f"thps{m % 3}", name="dtps")
            nc.tensor.matmul(ps[:], wdt_sb[:, m * P:(m + 1) * P],
                             dtT[:, th * 512:(th + 1) * 512],
                             start=True, stop=True)
            esc = p3b.tile([P, 512], F16, tag="esc")
            nc.scalar.activation(esc[:], ps[:], AF.Exp, bias=dtb_sb[:, m:m + 1])
            nc.scalar.activation(delta[:, m, th * 512:(th + 1) * 512],
                                 esc[:], AF.Ln, bias=1.0)

        def em_z(mt, th):
            ps = p2ps.tile([P, 512], F32, tag=f"thps{mt % 3}", name="zzps")
            for kd in range(4):
                nc.tensor.matmul(
                    ps[:], winz_sb[:, kd, mt * P:(mt + 1) * P],
                    xnT[:, kd, th * 512:(th + 1) * 512],
                    start=(kd == 0), stop=(kd == 3))
            nc.scalar.activation(zT[:, mt - 8, th * 512:(th + 1) * 512], ps[:],
                                 AF.Silu, bias=biasz_sb[:, mt:mt + 1])

        def em_toktail():
            # token-tail head (xn slice + cc/gate weight loads + transposes)
            ctx = tc.tile_wait_until(0.001 * TOKTAIL_WAIT)
            ctx.__enter__()
            xn_sl = layer_norm(xtok, 2, ptt, pttb,
                               gb=(bc["norm1_g"], bc["norm1_b"]),
                               out_dtype=F16, tag="lnsl")
            tt_tiles["xn_sl"] = xn_sl
            xnsT = ptt.tile([P, 4, 256], F16, tag="xnsT", name="xnsT")
            for tt in range(2):
                pst = pttps.tile([P, 4, P], F16, tag="ps6", name="ttps")
                for dd in range(4):
                    nc.tensor.transpose(pst[:, dd, :],
                                        xn_sl[:, tt, dd * P:(dd + 1) * P],
                                        idf16[:])
                nc.vector.tensor_copy(xnsT[:, :, tt * P:(tt + 1) * P], pst[:])
            tt_tiles["xnsT"] = xnsT
            ctx.__exit__(None, None, None)

        # th=0 chain inline: scan chunk 0 can start after this
        for mt in range(8):
            em_inconv(mt, 0)
        em_xproj(0)
        for m in range(4):
            em_delta(m, 0)
        with tc.tile_wait_until(0.040):
            for mt in range(8, 12):
                em_z(mt, 0)

        # deferred th=1 work, popped into the scan loop: one slot per (m, ngi)
        # iteration plus one per m-boundary (20 slots per chunk).  Hard
        # deadlines: conv(mt,1) all before xproj(1); delta(m,1) before chunk-1
        # iterations of m; z(mt,1) before chunk-1 ypost of its m.
        A_ = lambda mt: (lambda: em_inproj(mt, 1))
        B_ = lambda mt: (lambda: em_conv(mt, 1))
        Z_ = lambda mt: (lambda: em_z(mt, 1))
        D_ = lambda m: (lambda: em_delta(m, 1))
        deferred = [
            # ch0-m0 iters + end          # ch0-m1
            A_(0), A_(1), B_(0), A_(2), B_(1),
            A_(3), B_(2), A_(4), B_(3), A_(5),
            # ch0-m2                      # ch0-m3
            B_(4), A_(6), B_(5), A_(7), B_(6),
            B_(7), lambda: em_xproj(1), D_(0), D_(1), D_(2),
            # ch1-m0 iters + end
            D_(3), Z_(8), Z_(9), Z_(10), Z_(11),
            em_toktail,
        ]
        N_SLOTS = len(deferred)

        # token-tail x slice + small weights (DMA only; compute is deferred)
        tt_tiles = {}
        xtok = ptt.tile([P, 2, D], F32, tag="xtok")
        nc.sync.dma_start(xtok[:], di["x_tok"].ap().rearrange("(k p) d -> p k d", p=P))
        cw_sb = ptt.tile([P, 4, D], F16, tag="ccw")
        nc.gpsimd.dma_start(cw_sb[:], di["cc_wT"].ap().rearrange("(k p) m -> p k m", p=P))
        cnT_sb = ptt.tile([P, 4, NC_CLUST], F16, tag="cnT")
        nc.gpsimd.dma_start(cnT_sb[:], di["centers_nT"].ap().rearrange("(k p) m -> p k m", p=P))
        cent_sb = ptt.tile([NC_CLUST, D], F16, tag="cent")
        nc.gpsimd.dma_start(cent_sb[:], di["centers_dev"].ap())
        gw_sb = ptt.tile([P, 4, 2], F16, tag="gw")
        nc.gpsimd.dma_start(gw_sb[:], di["gate_wT"].ap().rearrange("(k p) m -> p k m", p=P))

        # ================= Phase 4+5: scan, y, out_proj ====================
        # n-sum strategy: pprod partials are accumulated over n on the PE
        # (identity-matmul into PSUM, idle during the scan), with D*xc folded
        # in as a diagonal matmul; dBu stays on DVE, pprod mostly on Pool.
        hprev = pw.tile([P, 4, NST], F16)
        dgD = pw.tile([P, 4, P], F16)       # diag(D) per m-block
        for m in range(4):
            nc.vector.tensor_scalar_mul(dgD[:, m, :], idf16[:], Dp_sb[:, m:m + 1])
        with tc.tile_pool(name="ph4", bufs=3) as p4, \
             tc.tile_pool(name="ph4bc", bufs=2) as p4bc, \
             tc.tile_pool(name="ph4da", bufs=2) as p4da, \
             tc.tile_pool(name="ph4y1", bufs=1) as p4y1, \
             tc.tile_pool(name="ph4ps", bufs=2, space="PSUM") as p4ps, \
             tc.tile_pool(name="ph5ps", bufs=1, space="PSUM") as p5ps:
            n_ch = L // TC
            for ch in range(n_ch):
                t0 = ch * TC
                yTf = p4y1.tile([P, 4, TC], F16, tag="yTf", name="yTf")
                dus = p4y1.tile([P, 4, TC], F16, tag="dus", name="dus")
                outT = p4y1.tile([P, 2, 2 * D], F16, tag="outT", name="outT")
                for m in range(4):
                    psy = p4ps.tile([P, TC], F32, tag="psy", name="psy")
                    nc.vector.tensor_mul(dus[:, m, :],
                                         delta[:, m, t0:t0 + TC],
                                         xcT[:, m, t0:t0 + TC])
                    for ngi in range(NST // NG):
                        nbase = ngi * NG
                        Bb = p4bc.tile([P, NG, TC], F16, tag="Bb")
                        nc.sync.dma_start(
                            Bb[:], bc_dram.ap()[None, nbase:nbase + NG, t0:t0 + TC]
                            .to_broadcast((P, NG, TC)))
                        Cb = p4bc.tile([P, NG, TC], F16, tag="Cb")
                        nc.sync.dma_start(
                            Cb[:], bc_dram.ap()[None, 16 + nbase:16 + nbase + NG, t0:t0 + TC]
                            .to_broadcast((P, NG, TC)))
                        dA = p4da.tile([P, NG, TC], F16, tag="dA")
                        for j in range(NG):
                            if a_vals is not None:
                                nc.scalar.activation(
                                    dA[:, j, :], delta[:, m, t0:t0 + TC], AF.Exp,
                                    scale=float(a_vals[nbase + j]))
                            else:
                                nc.scalar.activation(
                                    dA[:, j, :], delta[:, m, t0:t0 + TC], AF.Exp,
                                    scale=A_sb[:, m, nbase + j:nbase + j + 1])
                        if deferred:
                            deferred.pop(0)()
                        dBu = p4.tile([P, NG, TC], F16, tag="dBu")
                        nc.vector.tensor_tensor(
                            dBu[:], dus[:, m, None, :].to_broadcast((P, NG, TC)),
                            Bb[:], AL.mult)
                        h = p4.tile([P, NG, TC], F16, tag="h")
                        for j in range(NG):
                            init = 0.0 if ch == 0 else hprev[:, m, nbase + j:nbase + j + 1]
                            nc.vector.tensor_tensor_scan(
                                h[:, j, :], dA[:, j, :], dBu[:, j, :], init,
                                AL.mult, AL.add)
                        if ch < n_ch - 1:
                            nc.vector.tensor_copy(hprev[:, m, nbase:nbase + NG],
                                                  h[:, :, TC - 1])
                        pprod = p4.tile([P, NG, TC], F16, tag="pprod", name="pprod")
                        # pprod split DVE:Pool to unload DVE (the scan engine)
                        # while keeping the Pool link short
                        if PPROD_DVE_J > 0:
                            nc.vector.tensor_mul(pprod[:, 0:PPROD_DVE_J, :],
                                                 h[:, 0:PPROD_DVE_J, :],
                                                 Cb[:, 0:PPROD_DVE_J, :])
                        if PPROD_DVE_J < NG:
                            nc.gpsimd.tensor_mul(pprod[:, PPROD_DVE_J:, :],
                                                 h[:, PPROD_DVE_J:, :],
                                                 Cb[:, PPROD_DVE_J:, :])
                        # n-sum on PE: psy += sum_j pprod[:, j, :]
                        for j in range(NG):
                            nc.tensor.matmul(psy[:], idf16[:], pprod[:, j, :],
                                             start=(ngi == 0 and j == 0),
                                             stop=False)
                    # finish psum: += diag(D) @ xc, then gate with silu(z) + flip
                    nc.tensor.matmul(psy[:], dgD[:, m, :], xcT[:, m, t0:t0 + TC],
                                     start=False, stop=True)
                    yp_eng = nc.gpsimd if YPOST_POOL else nc.vector
                    if BUILD_NOIF:
                        yp_eng.tensor_tensor(yTf[:, m, :], psy[:],
                                             zT[:, m, t0:t0 + TC], AL.mult)
                    else:
                        pid = nc.partition_id()
                        with tc.If(pid >= 4) as cmp:
                            yp_eng.tensor_tensor(
                                yTf[:, m, :], psy[:, ::-1],
                                zT[:, m, t0:t0 + TC][:, ::-1], AL.mult)
                        with cmp.Else():
                            yp_eng.tensor_tensor(yTf[:, m, :], psy[:],
                                                 zT[:, m, t0:t0 + TC], AL.mult)
                    if deferred:
                        slot = N_SLOTS - len(deferred)
                        with tc.tile_wait_until(0.001 * (28 + 3.6 * slot)):
                            deferred.pop(0)()
                # out_proj (token-part output); for backward cores this chunk's
                # yTf holds true tokens [L-t0-TC, L-t0), i.e. chunk (n_ch-1-ch)
                for tt in range(4):
                    ps = p5ps.tile([P, 512], F32, tag="ops")
                    for m in range(4):
                        nc.tensor.matmul(ps[:], yTf[:, m, tt * P:(tt + 1) * P],
                                         wout_sb[:, m, :],
                                         start=(m == 0), stop=(m == 3))
                    nc.scalar.copy(outT[:, tt // 2, (tt % 2) * D:(tt % 2 + 1) * D], ps[:])
                pchs = [2 * ch, 2 * ch + 1]
                if BUILD_NOIF:
                    for p_ch in pchs:
                        for sub in range(2):
                            nc.sync.dma_start(
                                rs_in.ap()[p_ch, sub * P:(sub + 1) * P, :],
                                outT[:, p_ch - 2 * ch, sub * D:(sub + 1) * D])
                else:
                    with tc.If(pid >= 4) as cmp2:
                        for p_ch in pchs:
                            for sub in range(2):
                                nc.sync.dma_start(
                                    rs_in.ap()[p_ch ^ 2, sub * P:(sub + 1) * P, :],
                                    outT[:, p_ch - 2 * ch, sub * D:(sub + 1) * D])
                    with cmp2.Else():
                        for p_ch in pchs:
                            for sub in range(2):
                                nc.sync.dma_start(
                                    rs_in.ap()[p_ch, sub * P:(sub + 1) * P, :],
                                    outT[:, p_ch - 2 * ch, sub * D:(sub + 1) * D])

        early.close()

        # ====== Token-tail part 2: cc path, gate ====
        # (virtual release time keeps the greedy scheduler from hoisting these
        # Act/PE ops ahead of the scan-critical head chain)
        part2 = ExitStack()
        part2.enter_context(tc.tile_wait_until(0.001 * PART2_WAIT))
        xn_sl = tt_tiles["xn_sl"]
        xnsT = tt_tiles["xnsT"]
        projT = ptt.tile([P, 4, 256], F16, tag="projT")
        sqT = ptt.tile([P, 4, 256], F16, tag="sqT")
        for pf in range(4):
            ps = pttps.tile([P, 256], F32, tag="ps6")
            for kd in range(4):
                nc.tensor.matmul(ps[:], cw_sb[:, kd, pf * P:(pf + 1) * P],
                                 xnsT[:, kd, :], start=(kd == 0), stop=(kd == 3))
            nc.scalar.activation(projT[:, pf, :], ps[:], AF.Identity,
                                 bias=ccbias_sb[:, pf:pf + 1])
            nc.scalar.activation(sqT[:, pf, :], projT[:, pf, :], AF.Square)
        onescol = ptt.tile([P, 1], F16, tag="onescol")
        nc.vector.memset(onescol[:], 1.0)
        stack = ptt.tile([16, 256], F32, tag="stack")
        nc.vector.memset(stack[:], 0.0)
        ps_sim = pttps.tile([NC_CLUST, 256], F32, tag="pst6", name="ps_sim")
        for kd in range(4):
            nc.tensor.matmul(ps_sim[:], cnT_sb[:, kd, :], projT[:, kd, :],
                             start=(kd == 0), stop=(kd == 3))
        nc.scalar.copy(stack[0:8, :], ps_sim[:])
        ps_ssq = pttps.tile([1, 256], F32, tag="pst6", name="ps_ssq")
        for kd in range(4):
            nc.tensor.matmul(ps_ssq[:], onescol[:], sqT[:, kd, :],
                             start=(kd == 0), stop=(kd == 3))
        ssq_tmp = ptt.tile([1, 256], F32, tag="ssq_tmp")
        nc.scalar.copy(ssq_tmp[:], ps_ssq[:])
        nc.sync.dma_start(stack[8:9, :], ssq_tmp[:])
        S = ptt.tile([P, 2, 16], F32, tag="S")
        for tt in range(2):
            pst = pttps.tile([P, 16], F32, tag="pst6", name="stps")
            nc.tensor.transpose(pst[:], stack[:, tt * P:(tt + 1) * P],
                                idf32[:])
            nc.scalar.copy(S[:, tt, :], pst[:])
        nrm = ptt.tile([P, 2], F32, tag="nrm")
        nc.scalar.sqrt(nrm[:], S[:, :, 8])
        nc.vector.tensor_scalar_max(nrm[:], nrm[:], 1e-12)
        rnrm = ptt.tile([P, 2], F32, tag="rnrm")
        nc.vector.reciprocal(rnrm[:], nrm[:])
        wcl = ptt.tile([P, 2, NC_CLUST], F16, tag="wcl")
        for tt in range(2):
            sim = pttb.tile([P, NC_CLUST], F32, tag="sim")
            nc.vector.tensor_scalar_mul(sim[:], S[:, tt, 0:8], rnrm[:, tt:tt + 1])
            mx = pttb.tile([P, 1], F32, tag="mx")
            nc.vector.tensor_reduce(mx[:], sim[:], AX.X, AL.max)
            nmx = pttb.tile([P, 1], F32, tag="nmx")
            nc.vector.tensor_scalar_mul(nmx[:], mx[:], -1.0)
            se = pttb.tile([P, 1], F32, tag="se")
            ex = pttb.tile([P, NC_CLUST], F32, tag="ex")
            nc.scalar.activation(ex[:], sim[:], AF.Exp, bias=nmx[:], accum_out=se[:])
            rse = pttb.tile([P, 1], F32, tag="rse")
            nc.vector.reciprocal(rse[:], se[:])
            nc.vector.tensor_scalar_mul(wcl[:, tt, :], ex[:], rse[:])
        wclT = ptt.tile([NC_CLUST, 256], F16, tag="wclT")
        for tt in range(2):
            pst = pttps.tile([NC_CLUST, P], F16, tag="pst6", name="wtps")
            nc.tensor.transpose(pst[:], wcl[:, tt, :], idf16[:])
            nc.scalar.copy(wclT[:, tt * P:(tt + 1) * P], pst[:])
        ccpre = ptt.tile([P, 2, D], F32, tag="ccpre")
        for tt in range(2):
            ps = pttps.tile([P, D], F32, tag="ps6", name="ctxps")
            nc.tensor.matmul(ps[:], wclT[:, tt * P:(tt + 1) * P], cent_sb[:],
                             start=True, stop=True)
            nc.vector.scalar_tensor_tensor(ccpre[:, tt, :], ps[:], alpha_sb[:],
                                           xn_sl[:, tt, :], AL.mult, AL.add)
        cc_out = layer_norm(ccpre, 2, ptt, pttb, gb=(bc["ccg"], bc["ccb2"]),
                            out_dtype=F32, tag="lncc")

        gcl = ptt.tile([P, 2, 2], F32, tag="gcl")
        for tt in range(2):
            ps = pttps.tile([P, D], F32, tag="ps6", name="gps")
            for kd in range(4):
                nc.tensor.matmul(ps[:, 0:2], xnsT[:, kd, tt * P:(tt + 1) * P],
                                 gw_sb[:, kd, :], start=(kd == 0), stop=(kd == 3))
            gpre = pttb.tile([P, 2], F32, tag="gpre")
            nc.vector.tensor_add(gpre[:], ps[:, 0:2], bc["gate_b"][:])
            mx = pttb.tile([P, 1], F32, tag="gmx")
            nc.vector.tensor_reduce(mx[:], gpre[:], AX.X, AL.max)
            nmx = pttb.tile([P, 1], F32, tag="gnmx")
            nc.vector.tensor_scalar_mul(nmx[:], mx[:], -1.0)
            se = pttb.tile([P, 1], F32, tag="gse")
            ex = pttb.tile([P, 2], F32, tag="gex")
            nc.scalar.activation(ex[:], gpre[:], AF.Exp, bias=nmx[:], accum_out=se[:])
            rse = pttb.tile([P, 1], F32, tag="grse")
            nc.vector.reciprocal(rse[:], se[:])
            nc.vector.tensor_scalar_mul(gcl[:, tt, :], ex[:], rse[:])

        part2.close()
        if BUILD_NOCC:
            nc.sync.dma_start(rs_out.ap(), rs_in.ap()[0])
        else:
            nc.gpsimd.collective_compute(
                "ReduceScatter", AL.add, ins=[rs_in.ap()], outs=[rs_out.ap()],
                replica_groups=RG)
        mid.close()

        # ================= Late tail: fuse + FFN ===========================
        with tc.tile_pool(name="ph6", bufs=1) as p6, \
             tc.tile_pool(name="ph6b", bufs=2) as p6b, \
             tc.tile_pool(name="ph6ps", bufs=2, space="PSUM") as p6ps:
            mamba16 = p6.tile([P, 2, D], F16, tag="mamba16")
            nc.sync.dma_start(mamba16[:], rs_out.ap().rearrange("(k p) d -> p k d", p=P))
            mamba = p6.tile([P, 2, D], F32, tag="mamba")
            nc.vector.tensor_tensor(
                mamba[:], mamba16[:],
                bc["fusion_b"][:, None, :].to_broadcast((P, 2, D)), AL.add)

            # t0c precomputed (gcl/cc_out ready before the collective lands)
            t0c = p6.tile([P, 2, D], F32, tag="t0c")
            for tt in range(2):
                nc.vector.tensor_scalar_mul(t0c[:, tt, :], cc_out[:, tt, :],
                                            gcl[:, tt, 1:2])
                nc.vector.tensor_add(t0c[:, tt, :], t0c[:, tt, :], xtok[:, tt, :])
            x2 = p6.tile([P, 2, D], F32, tag="x2")
            for tt in range(2):
                nc.vector.scalar_tensor_tensor(x2[:, tt, :], mamba[:, tt, :],
                                               gcl[:, tt, 0:1], t0c[:, tt, :],
                                               AL.mult, AL.add)

            hln = layer_norm(x2, 2, p6, p6b, gb=None, out_dtype=F16, tag="lnffn")
            hT = p6.tile([P, 4, 256], F16, tag="hT")
            for tt in range(2):
                pst = p6ps.tile([P, 4, P], F16, tag="ps6", name="htps")
                for dd in range(4):
                    nc.tensor.transpose(pst[:, dd, :],
                                        hln[:, tt, dd * P:(dd + 1) * P],
                                        idf16[:])
                nc.vector.tensor_copy(hT[:, :, tt * P:(tt + 1) * P], pst[:])
            w1_sb = p6.tile([P, 4, 4 * D], F16, tag="w1")
            nc.gpsimd.dma_start(w1_sb[:], di["ffn_w1T"].ap().rearrange("(k p) m -> p k m", p=P))
            w2_sb = p6.tile([P, 16, D], F16, tag="w2")
            nc.gpsimd.dma_start(w2_sb[:], di["ffn_w2T"].ap().rearrange("(k p) m -> p k m", p=P))
            gT = p6.tile([P, 16, 256], F16, tag="gT")
            for gq in range(4):
                ps = p6ps.tile([P, 2, 256], F32, tag="ps6", name="f1ps")
                for gh in range(2):
                    gf = 2 * gq + gh
                    for kd in range(4):
                        nc.tensor.matmul(ps[:, gh, :],
                                         w1_sb[:, kd, gf * P:(gf + 1) * P],
                                         hT[:, kd, :], start=(kd == 0), stop=(kd == 3))
                nc.scalar.activation(gT[:, 2 * gq, :], ps[:, 0, :], AF.Gelu,
                                     bias=ffnb1_sb[:, 2 * gq:2 * gq + 1])
                nc.scalar.activation(gT[:, 2 * gq + 1, :], ps[:, 1, :], AF.Gelu,
                                     bias=ffnb1_sb[:, 2 * gq + 1:2 * gq + 2])
            for gq in range(4, 8):
                ps = p6ps.tile([P, 2, 256], F32, tag="ps6", name="f1ps")
                for gh in range(2):
                    gf = 2 * gq + gh
                    for kd in range(4):
                        nc.tensor.matmul(ps[:, gh, :],
                                         w1_sb[:, kd, gf * P:(gf + 1) * P],
                                         hT[:, kd, :], start=(kd == 0), stop=(kd == 3))
                nc.scalar.activation(gT[:, 2 * gq, :], ps[:, 0, :], AF.Gelu,
                                     bias=ffnb1_sb[:, 2 * gq:2 * gq + 1])
                nc.scalar.activation(gT[:, 2 * gq + 1, :], ps[:, 1, :], AF.Gelu,
                                     bias=ffnb1_sb[:, 2 * gq + 1:2 * gq + 2])
            for tt in range(2):
                ps = p6ps.tile([P, D], F32, tag="ps6", name="f2ps")
                for gf in range(16):
                    nc.tensor.matmul(ps[:], gT[:, gf, tt * P:(tt + 1) * P],
                                     w2_sb[:, gf, :], start=(gf == 0), stop=(gf == 15))
                ot = p6b.tile([P, D], F32, tag="ot")
                nc.vector.tensor_add(ot[:], ps[:], x2[:, tt, :])
                nc.vector.tensor_add(ot[:], ot[:], bc["ffn_b2"][:])
                nc.sync.dma_start(
                    out_slice.ap().rearrange("(k p) d -> p k d", p=P)[:, tt, :], ot[:])

    return nc


def _prep_inputs(inputs):
    """Build the 8 per-core input dicts from the full problem inputs."""
    x = _f32(inputs["x"])
    in_maps = []
    for c in range(N_CORES):
        half = c & 1
        batch = (c >> 1) & 1
        flip = c >= 4
        pos = (c & 1) + 2 * (c >> 2)
        pfx = "bm_" if flip else "fm_"
        g = lambda k: np.asarray(inputs[pfx + k])

        perm = np.r_[half * DH:(half + 1) * DH, (1 - half) * DH:(2 - half) * DH]
        in_w = np.asarray(g("in_w"))          # [2048, 512]
        xp_w = in_w[:DI][perm]
        z_w = in_w[DI + half * DH: DI + (half + 1) * DH]
        W_inz = np.concatenate([xp_w, z_w], axis=0)         # [1536, 512]
        n1g = _f32(inputs["norm1_g"])
        n1b = _f32(inputs["norm1_b"])
        wT_inz = _dt((W_inz * n1g[None, :]).T)
        bias_inz = _f32(W_inz @ n1b).reshape(12, P)

        xproj_w = np.asarray(g("xproj_w"))                  # [64, 1024]
        wT_xproj = _dt(xproj_w[:, perm].T)

        dt_w = np.asarray(g("dt_w"))                        # [1024, 32]
        wT_dt = _dt(dt_w[half * DH:(half + 1) * DH].T)
        dt_bias = _f32(g("dt_b")[half * DH:(half + 1) * DH]).reshape(4, P)

        A = -np.exp(_f32(g("A_log")))
        A_dev = _f32(A[half * DH:(half + 1) * DH])

        convw = _f32(g("conv_w")[:, 0, :][perm])
        convb = _f32(g("conv_b")[perm]).reshape(8, P)
        Dp_dev = _f32(g("D")[half * DH:(half + 1) * DH]).reshape(4, P)

        fusion_w = np.asarray(inputs["fusion_w"])
        # fusion input is concat(f_out, b_out): f -> cols 0:512, b -> 512:1024
        Wdir = fusion_w[:, 512:1024] if flip else fusion_w[:, 0:512]
        M = Wdir @ np.asarray(g("out_w"))                   # [512o, 1024di]
        wT_out = _dt(M[:, half * DH:(half + 1) * DH].T)

        centers = _f32(inputs["cc_centers"])
        cn = centers / np.maximum(np.linalg.norm(centers, axis=-1, keepdims=True), 1e-12)

        d = {
            "x_full": _f32(x[batch, ::-1] if flip else x[batch]),
            "x_tok": _f32(x[batch, pos * 256:(pos + 1) * 256]),
            "wT_inz": wT_inz,
            "bias_inz": bias_inz,
            "wT_xproj": wT_xproj,
            "wT_dt": wT_dt,
            "dt_bias": dt_bias,
            "A_dev": A_dev,
            "convw": convw,
            "convb": convb,
            "Dp_dev": Dp_dev,
            "wT_out": wT_out,
            "fusion_b": _f32(inputs["fusion_b"]).reshape(1, D),
            "cc_wT": _dt(np.asarray(inputs["cc_proj_w"]).T),
            "ccb": _f32(inputs["cc_proj_b"]).reshape(4, P),
            "centers_nT": _dt(cn.T),
            "centers_dev": _dt(centers),
            "norm1_g": n1g.reshape(1, D),
            "norm1_b": n1b.reshape(1, D),
            "ccg": _f32(inputs["cc_norm_g"]).reshape(1, D),
            "ccb2": _f32(inputs["cc_norm_b"]).reshape(1, D),
            "alpha_col": np.full((P, 1), float(np.asarray(inputs["cc_alpha"]).ravel()[0]), np.float32),
            "gate_wT": _dt(np.asarray(inputs["gate_w"]).T),
            "gate_b": _f32(inputs["gate_b"]).reshape(1, 2),
            "ffn_w1T": _dt((np.asarray(inputs["ffn_w1"]) * _f32(inputs["ffn_norm_g"])[None, :]).T),
            "ffn_b1": _f32(np.asarray(inputs["ffn_b1"]) + np.asarray(inputs["ffn_w1"]) @ _f32(inputs["ffn_norm_b"])).reshape(16, P),
            "ffn_w2T": _dt(np.asarray(inputs["ffn_w2"]).T),
            "ffn_b2": _f32(inputs["ffn_b2"]).reshape(1, D),
        }
        in_maps.append(d)
    return in_maps


TRACE = False
LAST_RESULT = {}


def _detect_uniform_A(inputs):
    As = [-np.exp(_f32(np.asarray(inputs[p + "A_log"]))) for p in ("fm_", "bm_")]
    a0 = As[0][0]
    for A in As:
        if not np.allclose(A, a0[None, :], rtol=0, atol=0):
            return None
    return tuple(float(v) for v in a0)


def kernel(**inputs):
    a_vals = _detect_uniform_A(inputs)
    key = ("nc", a_vals)
    if key not in _CACHED:
        nc = _build_nc(a_vals=a_vals)
        split_multi_waits(nc)
        _CACHED[key] = nc
    nc = _CACHED[key]
    in_maps = _prep_inputs(inputs)
    res = run_bass_kernel_spmd(nc, in_maps, core_ids=list(range(N_CORES)),
                               trace=TRACE)
    LAST_RESULT["res"] = res
    out = np.empty((2, L, D), np.float32)
    for c in range(N_CORES):
        batch = (c >> 1) & 1
        pos = (c & 1) + 2 * (c >> 2)
        out[batch, pos * 256:(pos + 1) * 256] = res.results[c]["out_slice"]
    return out

